# revision 28
# baseline (speedup 1.0000x reference)
"""AFNO transformer block on 8 Trainium2 NeuronCores.

Distribution:
  Phase 1 (channel-block sharded): core k owns channels [96k, 96k+96).
    LN1 stats partial sums -> per-batch AllReduce -> LN1 apply, then the
    whole spectral path (rFFT2 as DFT matmuls, block-diagonal complex MLP,
    inverse rFFT2) entirely core-local. DFTs use "flip" matmuls (data as
    the stationary operand) so every stage lands in the layout the next
    stage contracts over - no on-chip transposes.
  AllToAll (bf16 payload): filter output reshard (channel -> token).
  Phase 2 (token sharded): core j owns tokens [4050j, 4050j+4050).
    h = filt + LN1(x) + x assembled channel-major; LN1 per-token scales
    pre-broadcast to bf16 [128, 4050] planes via mask matmuls. LN2 stats
    by ones-matmul, normalize-first (htn = (h-m2)*r2 in bf16), then
    bf16 fc1 (g2-folded) -> exact Gelu -> bf16 fc2 -> residual. Weights
    for fc1 AND fc2 stay resident in SBUF in bf16. Output is written
    channel-major [C, TSH]; the host transposes.
"""
import math
import numpy as np
import ml_dtypes

import concourse.bass as bass
import concourse.mybir as mybir
import concourse.tile as tile
from concourse import bacc
from concourse.bass_utils import run_bass_kernel_spmd

F32 = mybir.dt.float32
BF16 = mybir.dt.bfloat16
AF = mybir.ActivationFunctionType
OP = mybir.AluOpType
AX = mybir.AxisListType

NCORES = 8
B, H, W, C = 2, 90, 180, 768
BS = 96           # channels per core / AFNO block size
KW = 46           # kept W-frequency modes
HID = 3072
LAM = 0.01
EPS = 1e-5
TOK = B * H * W   # 32400
TSH = TOK // NCORES  # 4050
NM = KW * H       # modes per batch elem: 4140
SQN = math.sqrt(H * W)

QB = [0, 506, 1519, 2532, 4050]      # a2a quarter boundaries (tokens)
TTS = [506, 507, 506, 507, 506, 506, 506, 506]
QMAP = [0, 1, 1, 2, 2, 3, 3, 3]      # chunk -> a2a quarter
TT0 = [sum(TTS[:i]) for i in range(len(TTS))]
NCH = len(TTS)
NCC = 6    # 768/128
WSCALE = 64.0  # fp8 weight pre-scale for fc1/fc2
NMO = 24   # 3072/128


def _dft_consts():
    wv = np.arange(W, dtype=np.float64)[:, None]
    wf = np.arange(KW, dtype=np.float64)[None, :]
    ang = 2.0 * np.pi * wv * wf / W
    fwr = np.cos(ang) / math.sqrt(W)
    fwi = -np.sin(ang) / math.sqrt(W)
    fwpack = np.concatenate([fwr, fwi], axis=1)          # (180, 92)
    hv = np.arange(H, dtype=np.float64)[:, None]
    hf = np.arange(H, dtype=np.float64)[None, :]
    angh = 2.0 * np.pi * hv * hf / H
    fhc = np.cos(angh) / math.sqrt(H)                    # symmetric
    fhs = np.sin(angh) / math.sqrt(H)
    alpha = np.ones(KW); alpha[1:] = 2.0
    iwr = alpha[None, :] * np.cos(ang) / math.sqrt(W)    # (180, 46)
    iwi = -alpha[None, :] * np.sin(ang) / math.sqrt(W)
    f32 = np.float32
    return (fwpack[:90].astype(f32), fwpack[90:].astype(f32),
            fhc.astype(f32), fhs.astype(f32), (-fhs).astype(f32),
            np.ascontiguousarray(iwr.T).astype(f32),
            np.ascontiguousarray(iwi.T).astype(f32))


def _shard_pieces(jq, lo=0, hi=TSH):
    """(ha,hb,wa,wb,tok_off) pieces of within-batch shard jq, tokens
    [lo, hi) of the shard; tok_off is relative to lo."""
    s0, e0 = TSH * jq + lo, TSH * jq + hi
    pieces, t = [], s0
    while t < e0:
        h = t // W
        wa = t - h * W
        if wa != 0 or e0 - t < W:
            wb = min(W, wa + (e0 - t))
            pieces.append((h, h + 1, wa, wb, t - s0))
            t += wb - wa
        else:
            hb = min(H, h + (e0 - t) // W)
            pieces.append((h, hb, 0, W, t - s0))
            t += (hb - h) * W
    return pieces


_CACHE = {}


def _build_nc():
    if "nc" in _CACHE:
        return _CACHE["nc"]
    nc = bacc.Bacc("TRN2", target_bir_lowering=False, debug=False,
                   num_devices=NCORES)
    g = lambda n, s, dt=F32: nc.dram_tensor(n, s, dt, kind="ExternalInput")
    xw = g("xw", [W, B, 2, H, 48], BF16)
    xc = g("xc", [C, TSH])
    fwp0 = g("fwp0", [90, 92], BF16); fwp1 = g("fwp1", [90, 92], BF16)
    fhc = g("fhc", [90, 90], BF16); fhs = g("fhs", [90, 90], BF16)
    fhsm = g("fhsm", [90, 90], BF16)
    iwrt = g("iwrt", [KW, W], BF16); iwit = g("iwit", [KW, W], BF16)
    w1r = g("w1r", [BS, BS], BF16); w1i = g("w1i", [BS, BS], BF16)
    w1im = g("w1im", [BS, BS], BF16)
    b1r = g("b1r", [BS, 1]); b1i = g("b1i", [BS, 1])
    w2a = g("w2a", [BS + 1, BS], BF16); w2b = g("w2b", [BS + 1, BS], BF16)
    w2c = g("w2c", [BS + 1, BS], BF16); w2d = g("w2d", [BS + 1, BS], BF16)
    g1col = g("g1col", [BS, 1]); spike = g("spike", [BS, 1])
    b2rr = g("b2rr", [1, BS], BF16); b2ir = g("b2ir", [1, BS], BF16)
    ones1b = g("ones1b", [1, 128], BF16)
    FP8 = mybir.dt.float8e4
    fc1q = g("fc1q", [3, 128, 2, HID], FP8)
    fc2q = g("fc2q", [12, 128, 2, C], FP8)
    gbias = g("gbias", [128, NMO])
    fc2b = g("fc2b", [128, NCC])
    g1f = g("g1f", [128, NCC]); be1f = g("be1f", [128, NCC])
    ones1 = g("ones1", [1, 128])
    ones128 = g("ones128", [128, 1])
    ones128b = g("ones128b", [128, 1], BF16)
    mask128 = g("mask128", [NCORES, 128])   # one-hot row = this core's shard

    out = nc.dram_tensor("out", [C, TSH], F32, kind="ExternalOutput")
    rg = [list(range(NCORES))]

    from contextlib import ExitStack
    with tile.TileContext(nc) as tc:
        with ExitStack() as _st0:
            cp = _st0.enter_context(tc.tile_pool(name="const", bufs=1))
            dram = _st0.enter_context(tc.tile_pool(name="dram", bufs=1, space="DRAM"))
            def cl(t, shape, dt=F32):
                nm = f"c_{t.name if hasattr(t, 'name') else t[:].tensor.name}"
                s = cp.tile(shape, dt, name=nm, tag=nm)
                nc.scalar.dma_start(s[:], t[:])
                return s
            c_fwp0 = cl(fwp0, [90, 92], BF16); c_fwp1 = cl(fwp1, [90, 92], BF16)
            c_fhc = cl(fhc, [90, 90], BF16); c_fhs = cl(fhs, [90, 90], BF16)
            c_fhsm = cl(fhsm, [90, 90], BF16)
            c_iwrt = cl(iwrt, [KW, W], BF16); c_iwit = cl(iwit, [KW, W], BF16)
            c_w1r = cl(w1r, [BS, BS], BF16); c_w1i = cl(w1i, [BS, BS], BF16)
            c_w1im = cl(w1im, [BS, BS], BF16)
            c_b1r = cl(b1r, [BS, 1]); c_b1i = cl(b1i, [BS, 1])
            c_w2a = cl(w2a, [BS + 1, BS], BF16); c_w2b = cl(w2b, [BS + 1, BS], BF16)
            c_w2c = cl(w2c, [BS + 1, BS], BF16); c_w2d = cl(w2d, [BS + 1, BS], BF16)
            c_g1col = cl(g1col, [BS, 1]); c_spike = cl(spike, [BS, 1])
            c_b2rr = cl(b2rr, [1, BS], BF16); c_b2ir = cl(b2ir, [1, BS], BF16)
            c_ones1b = cl(ones1b, [1, 128], BF16)
            c_ones1 = cl(ones1, [1, 128]); c_ones128 = cl(ones128, [128, 1])
            c_ones128b = cl(ones128b, [128, 1], BF16)
            c_gbias = cl(gbias, [128, NMO]); c_fc2b = cl(fc2b, [128, NCC])
            c_g1f = cl(g1f, [128, NCC]); c_be1f = cl(be1f, [128, NCC])
            c_mask = cl(mask128, [NCORES, 128])
            c_eps = cp.tile([128, 1], F32, name="c_eps")
            nc.vector.memset(c_eps[:], EPS)

            qws = [QB[q + 1] - QB[q] for q in range(4)]
            a2a_in = [dram.tile([NCORES, BS, qws[q]], BF16, name=f"a2ai{q}")
                      for q in range(4)]
            a2a_out = [dram.tile([NCORES, BS, qws[q]], BF16, name=f"a2ao{q}")
                       for q in range(4)]
            st_in = [dram.tile([2, W, H], F32, name=f"st_in{b_}") for b_ in range(B)]
            st_out = [dram.tile([2, W, H], F32, name=f"st_out{b_}") for b_ in range(B)]

            # ================= phase 1 =================
            with ExitStack() as _st1:
                stp = _st1.enter_context(tc.tile_pool(name="stats", bufs=1))
                zp = _st1.enter_context(tc.tile_pool(name="zp", bufs=1))
                lnt = _st1.enter_context(tc.tile_pool(name="lnt", bufs=2))
                spA = _st1.enter_context(tc.tile_pool(name="spA", bufs=1))
                spB = _st1.enter_context(tc.tile_pool(name="spB", bufs=1))
                spQ = _st1.enter_context(tc.tile_pool(name="spQ", bufs=1))
                sbg = _st1.enter_context(tc.tile_pool(name="sbg", bufs=1))
                clp = _st1.enter_context(tc.tile_pool(name="clipp", bufs=2))
                pp = _st1.enter_context(tc.tile_pool(name="psum1", bufs=8, space="PSUM"))

                s_sum = [stp.tile([90, 2, H], F32, tag=f"ss{b_}", name=f"ssum{b_}") for b_ in range(B)]
                s_sq = [stp.tile([90, 2, H], F32, tag=f"sq{b_}", name=f"ssq{b_}") for b_ in range(B)]
                s_m = [stp.tile([90, 2, H], F32, tag=f"sm{b_}", name=f"sm{b_}") for b_ in range(B)]
                s_r = [stp.tile([90, 2, H], F32, tag=f"sr{b_}", name=f"sr{b_}") for b_ in range(B)]
                s_mr = [stp.tile([90, 2, H], BF16, tag=f"smr{b_}", name=f"smr{b_}") for b_ in range(B)]
                s_t = stp.tile([90, H], F32, tag="st_tmp", name="s_tmp")

                # resident bf16 x tiles: zhs[(b, wc, ch)] = [90, H, 48]
                zhs = {}
                for b in range(B):
                    for wc in range(2):
                        for ch in range(2):
                            zh = zp.tile([90, H, 48], BF16, tag=f"z{b}{wc}{ch}",
                                         name=f"zh{b}{wc}{ch}")
                            nc.sync.dma_start(zh[:], xw[wc * 90:(wc + 1) * 90, b, ch])
                            zhs[(b, wc, ch)] = zh

                # ---- stats pass (both b) + AllReduce per b
                def _stk(t, kind):
                    return bass.AP(tensor=t[:].tensor,
                                   offset=t[:].offset + kind * W * H,
                                   ap=[[90, 90], [8100, 2], [1, 90]])

                def gp_reduce48(dst, src, eng):
                    # sum src [90, H, 48] over last axis into dst [90, H],
                    # as a halving tree on the given engine
                    tA = lnt.tile([90, H, 24], F32, tag="rA", name="rA")
                    eng.tensor_add(tA[:], src[:, :, 0:24], src[:, :, 24:48])
                    tB = lnt.tile([90, H, 12], F32, tag="rB", name="rB")
                    eng.tensor_add(tB[:], tA[:, :, 0:12], tA[:, :, 12:24])
                    tC = lnt.tile([90, H, 6], F32, tag="rC", name="rC")
                    eng.tensor_add(tC[:], tB[:, :, 0:6], tB[:, :, 6:12])
                    tD = lnt.tile([90, H, 3], F32, tag="rD", name="rD")
                    eng.tensor_add(tD[:], tC[:, :, 0:3], tC[:, :, 3:6])
                    eng.tensor_add(dst, tD[:, :, 0], tD[:, :, 1])
                    eng.tensor_add(dst, dst, tD[:, :, 2])

                for b in range(B):
                    for wc in range(2):
                        for ch in range(2):
                            zh = zhs[(b, wc, ch)]
                            sqh = spQ.tile([90, H, 48], F32, tag="QU", name="sqh")
                            nc.scalar.activation(out=sqh[:], in_=zh[:], func=AF.Square)
                            teng = nc.gpsimd if b == 0 else nc.vector
                            if ch == 0:
                                nc.vector.reduce_sum(s_sum[b][:, wc, :], zh[:], axis=AX.X)
                                gp_reduce48(s_sq[b][:, wc, :], sqh, teng)
                            else:
                                nc.vector.reduce_sum(s_t[:], zh[:], axis=AX.X)
                                nc.vector.tensor_add(s_sum[b][:, wc, :], s_sum[b][:, wc, :], s_t[:])
                                sq2 = stp.tile([90, H], F32, tag="st_tmp2", name="s_tmp2")
                                gp_reduce48(sq2[:], sqh, teng)
                                teng.tensor_add(s_sq[b][:, wc, :], s_sq[b][:, wc, :], sq2[:])
                    nc.sync.dma_start(_stk(st_in[b], 0), s_sum[b][:])
                    nc.sync.dma_start(_stk(st_in[b], 1), s_sq[b][:])
                    nc.gpsimd.collective_compute(
                        "AllReduce", OP.add, replica_groups=rg,
                        ins=[st_in[b][:].opt()], outs=[st_out[b][:].opt()])

                for b in range(B):
                    nc.sync.dma_start(s_sum[b][:], _stk(st_out[b], 0))
                    nc.sync.dma_start(s_sq[b][:], _stk(st_out[b], 1))
                    nc.vector.tensor_scalar(out=s_m[b][:], in0=s_sum[b][:],
                                            scalar1=1.0 / C, scalar2=None,
                                            op0=OP.mult)
                    nc.vector.tensor_scalar(out=s_r[b][:], in0=s_sq[b][:],
                                            scalar1=1.0 / C, scalar2=None,
                                            op0=OP.mult)
                    tmp = stp.tile([90, 2, H], F32, tag=f"tmp{b}", name=f"tmpb{b}")
                    nc.vector.tensor_mul(tmp[:], s_m[b][:], s_m[b][:])
                    nc.vector.tensor_sub(s_r[b][:], s_r[b][:], tmp[:])
                    nc.scalar.activation(out=s_r[b][:], in_=s_r[b][:],
                                         func=AF.Sqrt, bias=c_eps[:90])
                    nc.vector.reciprocal(s_r[b][:], s_r[b][:])
                    # m*r rows for the rank-1 DFT correction of the LN mean
                    nc.vector.tensor_mul(s_mr[b][:], s_m[b][:], s_r[b][:])

                def _bcast48(t, b, wc):
                    # [90, H] stat slice broadcast along a trailing 48-axis
                    base = t[b][:]
                    return bass.AP(tensor=base.tensor,
                                   offset=base.offset + wc * H,
                                   ap=[[base.ap[0][0], 90], [1, H], [0, 48]])

                for b in range(B):
                    # ---- LN1 scale-only (z *= r, in place); the mean term
                    # is a channel-independent rank-1 correction applied at
                    # the F1 drain: yb = F1(z*r) - DFT_w(m*r).
                    yb = spA.tile([90, 92, BS], BF16, tag="YO", name="yb")
                    mrd = lnt.tile([90, 92], F32, tag="mrd", name="mrd")
                    pmr = pp.tile([90, 92], F32, tag="pp", name="psmr")
                    nc.tensor.matmul(pmr[:], s_mr[b][:, 0, :], c_fwp0[:],
                                     start=True, stop=False)
                    nc.tensor.matmul(pmr[:], s_mr[b][:, 1, :], c_fwp1[:],
                                     start=False, stop=True)
                    nc.vector.tensor_copy(mrd[:], pmr[:])
                    for ch in range(2):
                        for wc in range(2):
                            zt = zhs[(b, wc, ch)]
                            nc.gpsimd.tensor_mul(zt[:], zt[:],
                                                 _bcast48(s_r, b, wc))
                        zh0 = zhs[(b, 0, ch)]
                        zh1 = zhs[(b, 1, ch)]
                        # 5 output columns packed per PSUM bank -> 5x fewer
                        # (and 5x bigger) drains, fused with the -m*r fixup
                        for gi, g0 in enumerate(range(0, 48, 5)):
                            ncol = min(5, 48 - g0)
                            ps = pp.tile([90, 460], F32, tag="pp", name="psf1")
                            for idx in range(ncol):
                                cl_ = g0 + idx
                                sl = ps[:, idx * 92:(idx + 1) * 92]
                                nc.tensor.matmul(sl, zh0[:, :, cl_], c_fwp0[:],
                                                 start=True, stop=False)
                                nc.tensor.matmul(sl, zh1[:, :, cl_], c_fwp1[:],
                                                 start=False, stop=True)
                            c0 = ch * 48 + g0
                            src = bass.AP(tensor=ps[:].tensor,
                                          offset=ps[:].offset,
                                          ap=[[ps[:].ap[0][0], 90],
                                              [1, 92], [92, ncol]])
                            mrdv = bass.AP(tensor=mrd[:].tensor,
                                           offset=mrd[:].offset,
                                           ap=[[mrd[:].ap[0][0], 90],
                                               [1, 92], [0, ncol]])
                            nc.vector.tensor_sub(yb[:, :, c0:c0 + ncol],
                                                 src, mrdv)

                    # ---- F2 (5 wf packed per PSUM bank)
                    zb = spB.tile([BS, 2, KW, H], BF16, tag="ZO", name="zbt")
                    for g0 in range(0, KW, 5):
                        nwf = min(5, KW - g0)
                        prg = pp.tile([BS, 450], F32, tag="pp", name="psf2r")
                        pig = pp.tile([BS, 450], F32, tag="pp", name="psf2i")
                        for idx in range(nwf):
                            wf = g0 + idx
                            yr = yb[:, wf, :]
                            yi = yb[:, 46 + wf, :]
                            slr = prg[:, idx * 90:(idx + 1) * 90]
                            nc.tensor.matmul(slr, yr, c_fhc[:], start=True, stop=False)
                            nc.tensor.matmul(slr, yi, c_fhs[:], start=False, stop=True)
                            sli = pig[:, idx * 90:(idx + 1) * 90]
                            nc.tensor.matmul(sli, yi, c_fhc[:], start=True, stop=False)
                            nc.tensor.matmul(sli, yr, c_fhsm[:], start=False, stop=True)
                        nc.scalar.activation(out=zb[:, 0, g0:g0 + nwf, :],
                                             in_=prg[:, :nwf * 90],
                                             func=AF.Copy, scale=c_g1col[:])
                        nc.scalar.activation(out=zb[:, 1, g0:g0 + nwf, :],
                                             in_=pig[:, :nwf * 90],
                                             func=AF.Copy, scale=c_g1col[:])
                    nc.vector.tensor_scalar(out=zb[:, 0, 0, 0:1],
                                            in0=zb[:, 0, 0, 0:1],
                                            scalar1=c_spike[:], scalar2=None,
                                            op0=OP.add)

                    # ---- block MLP layer 1
                    o1 = spA.tile([BS + 1, 2, NM], BF16, tag="YO", name="o1t")
                    zr_f = zb[:, 0].rearrange("p a b -> p (a b)")
                    zi_f = zb[:, 1].rearrange("p a b -> p (a b)")
                    n0 = 0
                    while n0 < NM:
                        nn_ = min(512, NM - n0)
                        zr_s = zr_f[:, n0:n0 + nn_]
                        zi_s = zi_f[:, n0:n0 + nn_]
                        por = pp.tile([BS, 512], F32, tag="pp", name="pso1r")
                        nc.tensor.matmul(por[:, :nn_], c_w1r[:], zr_s,
                                         start=True, stop=False)
                        nc.tensor.matmul(por[:, :nn_], c_w1im[:], zi_s,
                                         start=False, stop=True)
                        poi = pp.tile([BS, 512], F32, tag="pp", name="pso1i")
                        nc.tensor.matmul(poi[:, :nn_], c_w1i[:], zr_s,
                                         start=True, stop=False)
                        nc.tensor.matmul(poi[:, :nn_], c_w1r[:], zi_s,
                                         start=False, stop=True)
                        nc.scalar.activation(out=o1[0:BS, 0, n0:n0 + nn_],
                                             in_=por[:, :nn_], func=AF.Relu,
                                             bias=c_b1r[:])
                        nc.scalar.activation(out=o1[0:BS, 1, n0:n0 + nn_],
                                             in_=poi[:, :nn_], func=AF.Relu,
                                             bias=c_b1i[:])
                        n0 += nn_

                    # ---- block MLP layer 2 + softshrink (5 wf packed/bank)
                    o2 = spB.tile([H, 2, KW, BS], BF16, tag="ZO", name="o2t")
                    o1r_f = o1[:, 0]
                    o1i_f = o1[:, 1]
                    for g0 in range(0, KW, 5):
                        nwf = min(5, KW - g0)
                        prg = pp.tile([H, 480], F32, tag="pp", name="pso2r")
                        pig = pp.tile([H, 480], F32, tag="pp", name="pso2i")
                        for idx in range(nwf):
                            wf = g0 + idx
                            lr = o1r_f[0:BS, wf * H:(wf + 1) * H]
                            li = o1i_f[0:BS, wf * H:(wf + 1) * H]
                            slr = prg[:, idx * BS:(idx + 1) * BS]
                            nc.tensor.matmul(slr, lr, c_w2a[0:BS, :], start=True, stop=False)
                            nc.tensor.matmul(slr, li, c_w2b[0:BS, :], start=False, stop=False)
                            nc.tensor.matmul(slr, c_ones1b[:, 0:H], c_b2rr[:], start=False, stop=True)
                            sli = pig[:, idx * BS:(idx + 1) * BS]
                            nc.tensor.matmul(sli, li, c_w2c[0:BS, :], start=True, stop=False)
                            nc.tensor.matmul(sli, lr, c_w2d[0:BS, :], start=False, stop=False)
                            nc.tensor.matmul(sli, c_ones1b[:, 0:H], c_b2ir[:], start=False, stop=True)
                        for ri, psm in ((0, prg), (1, pig)):
                            clip = clp.tile([H, 480], F32, tag="clip", name="clipt")
                            nc.vector.tensor_scalar(out=clip[:, :nwf * BS],
                                                    in0=psm[:, :nwf * BS],
                                                    scalar1=-LAM, scalar2=LAM,
                                                    op0=OP.max, op1=OP.min)
                            nc.vector.tensor_sub(o2[:, ri, g0:g0 + nwf, :],
                                                 psm[:, :nwf * BS],
                                                 clip[:, :nwf * BS])

                    # ---- inverse H-DFT -> u2r/u2i [46, (c, h)] (5 c / bank)
                    u2r = spQ.tile([KW, BS, H], BF16, tag="QU", name="u2rt")
                    u2i = spA.tile([KW, BS, H], BF16, tag="YO", name="u2it")
                    for g0 in range(0, BS, 5):
                        ncl = min(5, BS - g0)
                        purg = pp.tile([KW, 450], F32, tag="pp", name="psur")
                        puig = pp.tile([KW, 450], F32, tag="pp", name="psui")
                        for idx in range(ncl):
                            c = g0 + idx
                            lr = o2[:, 0, :, c]
                            li = o2[:, 1, :, c]
                            slr = purg[:, idx * H:(idx + 1) * H]
                            nc.tensor.matmul(slr, lr, c_fhc[:], start=True, stop=False)
                            nc.tensor.matmul(slr, li, c_fhsm[:], start=False, stop=True)
                            sli = puig[:, idx * H:(idx + 1) * H]
                            nc.tensor.matmul(sli, li, c_fhc[:], start=True, stop=False)
                            nc.tensor.matmul(sli, lr, c_fhs[:], start=False, stop=True)
                        nc.scalar.activation(out=u2r[:, g0:g0 + ncl, :],
                                             in_=purg[:, :ncl * H], func=AF.Copy)
                        nc.vector.tensor_copy(u2i[:, g0:g0 + ncl, :],
                                              puig[:, :ncl * H])

                    # ---- inverse W-DFT -> SBUF gather sbA (bf16), 2 c/bank
                    sbA = sbg.tile([H, BS, W], BF16, tag="sbA", name="sbA")
                    for c0 in range(0, BS, 2):
                        pf = pp.tile([H, 360], F32, tag="pp", name="psf")
                        for idx in range(2):
                            c = c0 + idx
                            sl = pf[:, idx * W:(idx + 1) * W]
                            nc.tensor.matmul(sl, u2r[:, c, :], c_iwrt[:],
                                             start=True, stop=False)
                            nc.tensor.matmul(sl, u2i[:, c, :], c_iwit[:],
                                             start=False, stop=True)
                        if (c0 // 2) % 2 == 0:
                            nc.scalar.activation(out=sbA[:, c0:c0 + 2, :],
                                                 in_=pf[:], func=AF.Copy)
                        else:
                            nc.vector.tensor_copy(sbA[:, c0:c0 + 2, :], pf[:])

                    # ---- a2a send pieces (SBUF -> DRAM), quarter-major
                    # so quarter 0's collective can fire first
                    for q in range(4):
                        qw = qws[q]
                        for jq in range(4):
                            j = b * 4 + jq
                            for (ha, hb_, wa, wb_, toff) in _shard_pieces(
                                    jq, QB[q], QB[q + 1]):
                                src = sbA[ha:hb_, :, wa:wb_]
                                dst = bass.AP(
                                    tensor=a2a_in[q][:].tensor,
                                    offset=a2a_in[q][:].offset
                                    + (j * BS * qw + toff),
                                    ap=[[wb_ - wa, hb_ - ha], [qw, BS],
                                        [1, wb_ - wa]])
                                nc.sync.dma_start(dst, src)

            for q in range(4):
                nc.gpsimd.collective_compute(
                    "AllToAll", OP.bypass, replica_groups=rg,
                    ins=[a2a_in[q][:].opt()], outs=[a2a_out[q][:].opt()])

            # ================= phase 2 =================
            with ExitStack() as _st2:
                fc1p = _st2.enter_context(tc.tile_pool(name="fc1p", bufs=1))
                fc2p = _st2.enter_context(tc.tile_pool(name="fc2p", bufs=1))
                lnp = _st2.enter_context(tc.tile_pool(name="lnp", bufs=1))

                FP8 = mybir.dt.float8e4
                c_fc1 = [fc1p.tile([128, 2, HID], FP8, tag=f"fc1_{i}",
                                   name=f"cfc1_{i}") for i in range(3)]
                for i in range(3):
                    nc.sync.dma_start(c_fc1[i][:], fc1q[i])
                c_fc2 = [fc2p.tile([128, 2, C], FP8, tag=f"fc2_{i}",
                                   name=f"cfc2_{i}") for i in range(12)]
                for i in range(12):
                    nc.sync.dma_start(c_fc2[i][:], fc2q[i])

                R1B = lnp.tile([128, TSH], BF16, tag="R1B", name="R1B")
                MR1B = lnp.tile([128, TSH], BF16, tag="MR1B", name="MR1B")

                with ExitStack() as _stR:
                    rowp = _stR.enter_context(tc.tile_pool(name="rowp", bufs=1))
                    pbR = _stR.enter_context(
                        tc.tile_pool(name="pbR", bufs=2, space="PSUM"))
                    # all-shard LN1 stats rows [8, 4050], computed in place
                    r1_8 = rowp.tile([NCORES, TSH], F32, tag="r18", name="r18")
                    mr1_8 = rowp.tile([NCORES, TSH], F32, tag="mr18", name="mr18")
                    rtmp = rowp.tile([NCORES, TSH], F32, tag="rtmp", name="rtmp")
                    for kind, dstt in ((0, rtmp), (1, r1_8)):
                        for s in range(NCORES):
                            bb, jq = s // 4, s % 4
                            for (ha, hb_, wa, wb_, toff) in _shard_pieces(jq):
                                src_ = bass.AP(
                                    tensor=st_out[bb][:].tensor,
                                    offset=st_out[bb][:].offset
                                    + (kind * W * H + wa * H + ha),
                                    ap=[[0, 1], [1, hb_ - ha], [H, wb_ - wa]])
                                nc.sync.dma_start(
                                    dstt[s:s + 1,
                                         toff:toff + (hb_ - ha) * (wb_ - wa)],
                                    src_)
                    nc.vector.tensor_scalar(out=mr1_8[:], in0=rtmp[:],
                                            scalar1=1.0 / C, scalar2=None,
                                            op0=OP.mult)           # m1
                    nc.vector.tensor_scalar(out=r1_8[:], in0=r1_8[:],
                                            scalar1=1.0 / C, scalar2=None,
                                            op0=OP.mult)           # q/C
                    nc.vector.tensor_mul(rtmp[:], mr1_8[:], mr1_8[:])
                    nc.vector.tensor_sub(r1_8[:], r1_8[:], rtmp[:])  # var
                    nc.scalar.activation(out=r1_8[:], in_=r1_8[:], func=AF.Sqrt,
                                         bias=c_eps[:NCORES])
                    nc.vector.reciprocal(r1_8[:], r1_8[:])           # r1
                    nc.vector.tensor_mul(mr1_8[:], mr1_8[:], r1_8[:])  # m1*r1

                    # pre-broadcast this core's r1 / m1*r1 to bf16 planes
                    for it, T in enumerate(TTS):
                        t0 = TT0[it]
                        for rows, plane in ((r1_8, R1B), (mr1_8, MR1B)):
                            pb = pbR.tile([128, 512], F32, tag="pbc")
                            nc.tensor.matmul(pb[:, :T], c_mask[:],
                                             rows[:, t0:t0 + T],
                                             start=True, stop=True)
                            nc.scalar.activation(out=plane[:, t0:t0 + T],
                                                 in_=pb[:, :T], func=AF.Copy)

                xcp = _st2.enter_context(tc.tile_pool(name="xcp", bufs=6))
                rcvp = _st2.enter_context(tc.tile_pool(name="rcvp", bufs=6))
                t1p = _st2.enter_context(tc.tile_pool(name="t1p", bufs=3))
                htokp = _st2.enter_context(tc.tile_pool(name="htokp", bufs=12))
                htnp = _st2.enter_context(tc.tile_pool(name="htnp", bufs=12))
                hidp = _st2.enter_context(tc.tile_pool(name="hidp", bufs=1))
                rw2 = _st2.enter_context(tc.tile_pool(name="rw2", bufs=2))
                bcp = _st2.enter_context(tc.tile_pool(name="bcp", bufs=2))
                outp = _st2.enter_context(tc.tile_pool(name="outp", bufs=3))
                ph = _st2.enter_context(tc.tile_pool(name="ph", bufs=2, space="PSUM"))
                po = _st2.enter_context(tc.tile_pool(name="po", bufs=2, space="PSUM"))
                pst = _st2.enter_context(tc.tile_pool(name="pst", bufs=2, space="PSUM"))
                pbc = _st2.enter_context(tc.tile_pool(name="pbc", bufs=2, space="PSUM"))

                # software-pipelined chunk loop: front half (loads, assemble,
                # LN2 stats, normalize) runs one chunk ahead of the back half
                # (fc1 -> gelu -> fc2 -> residual -> store).
                hts = {}
                htns = {}
                hid = hidp.tile([128, NMO, 512], FP8, tag="hid", name="hid")
                for i in range(NCH + 1):
                    if i < NCH:
                        T = TTS[i]
                        t0 = TT0[i]
                        htoks = []
                        htnl = []
                        for cc in range(NCC):
                            xct = xcp.tile([128, 512], F32, tag="xct")
                            nc.sync.dma_start(xct[:, :T],
                                              xc[cc * 128:(cc + 1) * 128, t0:t0 + T])
                            rcv = rcvp.tile([128, 512], BF16, tag="rcv")
                            q = QMAP[i]
                            lt0 = t0 - QB[q]
                            c0 = cc * 128
                            r0 = 0
                            while r0 < 128:
                                s_blk = (c0 + r0) // BS
                                c_in = (c0 + r0) % BS
                                nrow = min(BS - c_in, 128 - r0)
                                nc.sync.dma_start(
                                    rcv[r0:r0 + nrow, :T],
                                    a2a_out[q][s_blk, c_in:c_in + nrow,
                                               lt0:lt0 + T])
                                r0 += nrow
                            ht = htokp.tile([128, 512], F32, tag="htok")
                            htoks.append(ht)
                            t1 = t1p.tile([128, 512], F32, tag="t1")
                            nc.gpsimd.tensor_mul(t1[:, :T], xct[:, :T],
                                                 R1B[:, t0:t0 + T])
                            nc.vector.tensor_sub(t1[:, :T], t1[:, :T],
                                                 MR1B[:, t0:t0 + T])
                            nc.vector.tensor_scalar(out=t1[:, :T], in0=t1[:, :T],
                                                    scalar1=c_g1f[:, cc:cc + 1],
                                                    scalar2=c_be1f[:, cc:cc + 1],
                                                    op0=OP.mult, op1=OP.add)
                            nc.gpsimd.tensor_add(ht[:, :T], rcv[:, :T], xct[:, :T])
                            nc.vector.tensor_add(ht[:, :T], ht[:, :T], t1[:, :T])

                        # LN2 stats via bf16 ones-matmul (1 cyc/row vs 4)
                        ps_s = pst.tile([1, 512], F32, tag="pst")
                        ps_q = pst.tile([1, 512], F32, tag="pst")
                        htbs = []
                        for cc in range(NCC):
                            htb = t1p.tile([128, 512], BF16, tag="htb", name="htb")
                            nc.scalar.activation(out=htb[:, :T],
                                                 in_=htoks[cc][:, :T],
                                                 func=AF.Copy)
                            htbs.append(htb)
                        for cc in range(NCC):
                            nc.tensor.matmul(ps_s[:, :T], c_ones128b[:],
                                             htbs[cc][:, :T],
                                             start=(cc == 0), stop=(cc == NCC - 1))
                        hsqs = []
                        for cc in range(NCC):
                            hsq = t1p.tile([128, 512], BF16, tag="hsq", name="hsq")
                            nc.gpsimd.tensor_mul(hsq[:, :T], htbs[cc][:, :T],
                                                 htbs[cc][:, :T])
                            hsqs.append(hsq)
                        for cc in range(NCC):
                            nc.tensor.matmul(ps_q[:, :T], c_ones128b[:],
                                             hsqs[cc][:, :T],
                                             start=(cc == 0), stop=(cc == NCC - 1))
                        m2r = rw2.tile([1, 512], F32, tag="m2r")
                        r2r = rw2.tile([1, 512], F32, tag="r2r")
                        vv = rw2.tile([1, 512], F32, tag="vv")
                        nc.vector.tensor_scalar(out=m2r[:, :T], in0=ps_s[:, :T],
                                                scalar1=1.0 / C, scalar2=None,
                                                op0=OP.mult)
                        nc.vector.tensor_scalar(out=r2r[:, :T], in0=ps_q[:, :T],
                                                scalar1=1.0 / C, scalar2=None,
                                                op0=OP.mult)
                        nc.vector.tensor_mul(vv[:, :T], m2r[:, :T], m2r[:, :T])
                        nc.vector.tensor_sub(r2r[:, :T], r2r[:, :T], vv[:, :T])
                        nc.scalar.activation(out=r2r[:, :T], in_=r2r[:, :T],
                                             func=AF.Sqrt, bias=c_eps[:1])
                        nc.vector.reciprocal(r2r[:, :T], r2r[:, :T])
                        # broadcast m2, r2 to all partitions
                        m2b = bcp.tile([128, 512], F32, tag="m2b")
                        r2b = bcp.tile([128, 512], F32, tag="r2b")
                        for rowt, bt in ((m2r, m2b), (r2r, r2b)):
                            pb = pbc.tile([128, 512], F32, tag="pbc")
                            nc.tensor.matmul(pb[:, :T], c_ones1[:], rowt[:, :T],
                                             start=True, stop=True)
                            nc.scalar.activation(out=bt[:, :T], in_=pb[:, :T],
                                                 func=AF.Copy)
                        # normalize -> fp8e4m3 (DoubleRow slot layout)
                        for cc in range(NCC):
                            if cc % 2 == 0:
                                htn = htnp.tile([128, 2, 512], FP8, tag="htn")
                                htnl.append(htn)
                            tn = t1p.tile([128, 512], F32, tag="t1", name="tn")
                            nc.vector.tensor_sub(tn[:, :T], htoks[cc][:, :T],
                                                 m2b[:, :T])
                            nc.vector.tensor_mul(htn[:, cc % 2, :T], tn[:, :T],
                                                 r2b[:, :T])
                        hts[i] = htoks
                        htns[i] = htnl

                    if i >= 1:
                        j = i - 1
                        T = TTS[j]
                        t0 = TT0[j]
                        htoks = hts.pop(j)
                        htnl = htns.pop(j)
                        # fc1 (fp8 DoubleRow) + gelu -> hid (fp8)
                        for mo in range(NMO):
                            php = ph.tile([128, 512], F32, tag="ph")
                            for p_ in range(3):
                                nc.tensor.matmul(
                                    php[:, :T],
                                    c_fc1[p_][:, :, mo * 128:(mo + 1) * 128],
                                    htnl[p_][:, :, :T],
                                    start=(p_ == 0), stop=(p_ == 2),
                                    perf_mode=mybir.MatmulPerfMode.DoubleRow)
                            nc.scalar.activation(out=hid[:, mo, :T],
                                                 in_=php[:, :T], func=AF.Gelu,
                                                 scale=1.0 / WSCALE,
                                                 bias=c_gbias[:, mo:mo + 1])
                        # fc2 (fp8 DoubleRow) + bias + residual -> store
                        for co in range(NCC):
                            pop = po.tile([128, 512], F32, tag="po")
                            for p_ in range(12):
                                nc.tensor.matmul(
                                    pop[:, :T],
                                    c_fc2[p_][:, :, co * 128:(co + 1) * 128],
                                    hid[:, 2 * p_:2 * p_ + 2, :T],
                                    start=(p_ == 0), stop=(p_ == 11),
                                    perf_mode=mybir.MatmulPerfMode.DoubleRow)
                            osb = outp.tile([128, 512], F32, tag="osb")
                            nc.scalar.activation(out=osb[:, :T],
                                                 in_=pop[:, :T],
                                                 func=AF.Identity,
                                                 scale=1.0 / WSCALE,
                                                 bias=c_fc2b[:, co:co + 1])
                            nc.vector.tensor_add(osb[:, :T], osb[:, :T],
                                                 htoks[co][:, :T])
                            nc.sync.dma_start(
                                out[co * 128:(co + 1) * 128, t0:t0 + T],
                                osb[:, :T])

    nc.compile()
    _CACHE["nc"] = nc
    return nc


def _host_prep(inputs):
    x = np.ascontiguousarray(np.asarray(inputs["x"], dtype=np.float32))
    g1 = np.asarray(inputs["g1"], np.float32); be1 = np.asarray(inputs["be1"], np.float32)
    g2 = np.asarray(inputs["g2"], np.float32); be2 = np.asarray(inputs["be2"], np.float32)
    w1 = np.asarray(inputs["w1"], np.float32); b1 = np.asarray(inputs["b1"], np.float32)
    w2 = np.asarray(inputs["w2"], np.float32); b2 = np.asarray(inputs["b2"], np.float32)
    fc1_w = np.asarray(inputs["fc1_w"], np.float32)
    fc1_b = np.asarray(inputs["fc1_b"], np.float32)
    fc2_w = np.asarray(inputs["fc2_w"], np.float32)
    fc2_b = np.asarray(inputs["fc2_b"], np.float32)

    fwp0, fwp1, fhc_m, fhs_m, fhsm_m, iwrt_m, iwit_m = _dft_consts()
    xf = x.reshape(TOK, C)
    bf = ml_dtypes.bfloat16
    f8 = ml_dtypes.float8_e4m3fn
    fc1q_m = np.ascontiguousarray(
        (g2[:, None] * fc1_w * WSCALE).reshape(3, 2, 128, HID)
        .transpose(0, 2, 1, 3)).astype(f8)                     # (3,128,2,3072)
    fc2q_m = np.ascontiguousarray(
        (fc2_w * WSCALE).reshape(12, 2, 128, C)
        .transpose(0, 2, 1, 3)).astype(f8)                     # (12,128,2,768)
    gbias_v = (fc1_b + be2 @ fc1_w).astype(np.float32)         # (3072,)
    gbias_m = np.ascontiguousarray(gbias_v.reshape(NMO, 128).T)  # (128, 24)
    fc2b_m = np.ascontiguousarray(fc2_b.reshape(NCC, 128).T)
    g1f_m = np.ascontiguousarray(g1.reshape(NCC, 128).T)
    be1f_m = np.ascontiguousarray(be1.reshape(NCC, 128).T)
    ones1 = np.ones((1, 128), np.float32)
    ones1b = np.ones((1, 128), bf)
    ones128 = np.ones((128, 1), np.float32)
    ones128b = np.ones((128, 1), bf)

    in_maps = []
    for k in range(NCORES):
        ck = slice(k * BS, (k + 1) * BS)
        # (W, B, H, 96) -> (W, B, 2, H, 48), bf16
        xw_k = np.ascontiguousarray(
            x[:, :, :, ck].transpose(2, 0, 1, 3)
            .reshape(W, B, H, 2, 48).transpose(0, 1, 3, 2, 4)).astype(bf)
        xc_k = np.ascontiguousarray(xf[k * TSH:(k + 1) * TSH, :].T)
        w1r_k = np.ascontiguousarray(w1[k, :, :, 0])
        w1i_k = np.ascontiguousarray(w1[k, :, :, 1])
        w2r_k = np.ascontiguousarray(w2[k, :, :, 0])
        w2i_k = np.ascontiguousarray(w2[k, :, :, 1])
        b2r_k = b2[k, :, 0]; b2i_k = b2[k, :, 1]
        zr = np.zeros((1, BS), np.float32)
        mask = np.zeros((NCORES, 128), np.float32); mask[k, :] = 1.0
        in_maps.append({
            "xw": xw_k, "xc": xc_k,
            "fwp0": fwp0.astype(bf), "fwp1": fwp1.astype(bf),
            "fhc": fhc_m.astype(bf), "fhs": fhs_m.astype(bf),
            "fhsm": fhsm_m.astype(bf),
            "iwrt": iwrt_m.astype(bf), "iwit": iwit_m.astype(bf),
            "w1r": w1r_k.astype(bf), "w1i": w1i_k.astype(bf),
            "w1im": (-w1i_k).astype(bf),
            "b1r": b1[k, :, 0:1].copy(), "b1i": b1[k, :, 1:2].copy(),
            "w2a": np.concatenate([w2r_k, b2r_k[None, :]], 0).astype(bf),
            "w2b": np.concatenate([-w2i_k, zr], 0).astype(bf),
            "w2c": np.concatenate([w2r_k, zr], 0).astype(bf),
            "w2d": np.concatenate([w2i_k, b2i_k[None, :]], 0).astype(bf),
            "g1col": g1[ck][:, None].copy(),
            "b2rr": b2r_k[None, :].astype(bf), "b2ir": b2i_k[None, :].astype(bf),
            "spike": (be1[ck] * SQN)[:, None].astype(np.float32),
            "fc1q": fc1q_m, "fc2q": fc2q_m, "gbias": gbias_m,
            "fc2b": fc2b_m, "g1f": g1f_m, "be1f": be1f_m,
            "ones1": ones1, "ones1b": ones1b, "ones128": ones128,
            "ones128b": ones128b,
            "mask128": mask,
        })
    return in_maps


def kernel(**inputs):
    nc = _build_nc()
    in_maps = _host_prep(inputs)
    res = run_bass_kernel_spmd(nc, in_maps, core_ids=list(range(NCORES)))
    outs = [np.asarray(res.results[j]["out"], dtype=np.float32).T
            for j in range(NCORES)]
    full = np.concatenate(outs, axis=0).reshape(B, H, W, C)
    return np.ascontiguousarray(full, dtype=np.float32)


# revision 30
# speedup vs baseline: 1.0085x; 1.0085x over previous
"""AFNO transformer block on 8 Trainium2 NeuronCores.

Distribution:
  Phase 1 (channel-block sharded): core k owns channels [96k, 96k+96).
    LN1 stats partial sums -> per-batch AllReduce -> LN1 apply, then the
    whole spectral path (rFFT2 as DFT matmuls, block-diagonal complex MLP,
    inverse rFFT2) entirely core-local. DFTs use "flip" matmuls (data as
    the stationary operand) so every stage lands in the layout the next
    stage contracts over - no on-chip transposes.
  AllToAll (bf16 payload): filter output reshard (channel -> token).
  Phase 2 (token sharded): core j owns tokens [4050j, 4050j+4050).
    h = filt + LN1(x) + x assembled channel-major; LN1 per-token scales
    pre-broadcast to bf16 [128, 4050] planes via mask matmuls. LN2 stats
    by ones-matmul, normalize-first (htn = (h-m2)*r2 in bf16), then
    bf16 fc1 (g2-folded) -> exact Gelu -> bf16 fc2 -> residual. Weights
    for fc1 AND fc2 stay resident in SBUF in bf16. Output is written
    channel-major [C, TSH]; the host transposes.
"""
import math
import numpy as np
import ml_dtypes

import concourse.bass as bass
import concourse.mybir as mybir
import concourse.tile as tile
from concourse import bacc
from concourse.bass_utils import run_bass_kernel_spmd

F32 = mybir.dt.float32
BF16 = mybir.dt.bfloat16
AF = mybir.ActivationFunctionType
OP = mybir.AluOpType
AX = mybir.AxisListType

NCORES = 8
B, H, W, C = 2, 90, 180, 768
BS = 96           # channels per core / AFNO block size
KW = 46           # kept W-frequency modes
HID = 3072
LAM = 0.01
EPS = 1e-5
TOK = B * H * W   # 32400
TSH = TOK // NCORES  # 4050
NM = KW * H       # modes per batch elem: 4140
SQN = math.sqrt(H * W)

QB = [0, 506, 1519, 2532, 4050]      # a2a quarter boundaries (tokens)
TTS = [506, 507, 506, 507, 506, 506, 506, 506]
QMAP = [0, 1, 1, 2, 2, 3, 3, 3]      # chunk -> a2a quarter
TT0 = [sum(TTS[:i]) for i in range(len(TTS))]
NCH = len(TTS)
NCC = 6    # 768/128
WSCALE = 64.0  # fp8 weight pre-scale for fc1/fc2
NMO = 24   # 3072/128


def _dft_consts():
    wv = np.arange(W, dtype=np.float64)[:, None]
    wf = np.arange(KW, dtype=np.float64)[None, :]
    ang = 2.0 * np.pi * wv * wf / W
    fwr = np.cos(ang) / math.sqrt(W)
    fwi = -np.sin(ang) / math.sqrt(W)
    fwpack = np.concatenate([fwr, fwi], axis=1)          # (180, 92)
    hv = np.arange(H, dtype=np.float64)[:, None]
    hf = np.arange(H, dtype=np.float64)[None, :]
    angh = 2.0 * np.pi * hv * hf / H
    fhc = np.cos(angh) / math.sqrt(H)                    # symmetric
    fhs = np.sin(angh) / math.sqrt(H)
    alpha = np.ones(KW); alpha[1:] = 2.0
    iwr = alpha[None, :] * np.cos(ang) / math.sqrt(W)    # (180, 46)
    iwi = -alpha[None, :] * np.sin(ang) / math.sqrt(W)
    f32 = np.float32
    return (fwpack[:90].astype(f32), fwpack[90:].astype(f32),
            fhc.astype(f32), fhs.astype(f32), (-fhs).astype(f32),
            np.ascontiguousarray(iwr.T).astype(f32),
            np.ascontiguousarray(iwi.T).astype(f32))


def _shard_pieces(jq, lo=0, hi=TSH):
    """(ha,hb,wa,wb,tok_off) pieces of within-batch shard jq, tokens
    [lo, hi) of the shard; tok_off is relative to lo."""
    s0, e0 = TSH * jq + lo, TSH * jq + hi
    pieces, t = [], s0
    while t < e0:
        h = t // W
        wa = t - h * W
        if wa != 0 or e0 - t < W:
            wb = min(W, wa + (e0 - t))
            pieces.append((h, h + 1, wa, wb, t - s0))
            t += wb - wa
        else:
            hb = min(H, h + (e0 - t) // W)
            pieces.append((h, hb, 0, W, t - s0))
            t += (hb - h) * W
    return pieces


_CACHE = {}


def _build_nc():
    if "nc" in _CACHE:
        return _CACHE["nc"]
    nc = bacc.Bacc("TRN2", target_bir_lowering=False, debug=False,
                   num_devices=NCORES)
    g = lambda n, s, dt=F32: nc.dram_tensor(n, s, dt, kind="ExternalInput")
    xw = g("xw", [W, B, 2, H, 48], BF16)
    xc = g("xc", [C, TSH])
    fwp0 = g("fwp0", [90, 92], BF16); fwp1 = g("fwp1", [90, 92], BF16)
    fhc = g("fhc", [90, 90], BF16); fhs = g("fhs", [90, 90], BF16)
    fhsm = g("fhsm", [90, 90], BF16)
    iwrt = g("iwrt", [KW, W], BF16); iwit = g("iwit", [KW, W], BF16)
    w1r = g("w1r", [BS, BS], BF16); w1i = g("w1i", [BS, BS], BF16)
    w1im = g("w1im", [BS, BS], BF16)
    b1r = g("b1r", [BS, 1]); b1i = g("b1i", [BS, 1])
    w2a = g("w2a", [BS + 1, BS], BF16); w2b = g("w2b", [BS + 1, BS], BF16)
    w2c = g("w2c", [BS + 1, BS], BF16); w2d = g("w2d", [BS + 1, BS], BF16)
    g1col = g("g1col", [BS, 1]); spike = g("spike", [BS, 1])
    b2rr = g("b2rr", [1, BS], BF16); b2ir = g("b2ir", [1, BS], BF16)
    ones1b = g("ones1b", [1, 128], BF16)
    FP8 = mybir.dt.float8e4
    fc1q = g("fc1q", [3, 128, 2, HID], FP8)
    fc2q = g("fc2q", [12, 128, 2, C], FP8)
    gbias = g("gbias", [128, NMO])
    fc2b = g("fc2b", [128, NCC])
    g1f = g("g1f", [128, NCC]); be1f = g("be1f", [128, NCC])
    ones1 = g("ones1", [1, 128])
    ones128 = g("ones128", [128, 1])
    ones128b = g("ones128b", [128, 1], BF16)
    mask128 = g("mask128", [NCORES, 128])   # one-hot row = this core's shard

    out = nc.dram_tensor("out", [C, TSH], F32, kind="ExternalOutput")
    rg = [list(range(NCORES))]

    from contextlib import ExitStack
    with tile.TileContext(nc) as tc:
        with ExitStack() as _st0:
            cp = _st0.enter_context(tc.tile_pool(name="const", bufs=1))
            dram = _st0.enter_context(tc.tile_pool(name="dram", bufs=1, space="DRAM"))
            def cl(t, shape, dt=F32):
                nm = f"c_{t.name if hasattr(t, 'name') else t[:].tensor.name}"
                s = cp.tile(shape, dt, name=nm, tag=nm)
                nc.scalar.dma_start(s[:], t[:])
                return s
            c_fwp0 = cl(fwp0, [90, 92], BF16); c_fwp1 = cl(fwp1, [90, 92], BF16)
            c_fhc = cl(fhc, [90, 90], BF16); c_fhs = cl(fhs, [90, 90], BF16)
            c_fhsm = cl(fhsm, [90, 90], BF16)
            c_iwrt = cl(iwrt, [KW, W], BF16); c_iwit = cl(iwit, [KW, W], BF16)
            c_w1r = cl(w1r, [BS, BS], BF16); c_w1i = cl(w1i, [BS, BS], BF16)
            c_w1im = cl(w1im, [BS, BS], BF16)
            c_b1r = cl(b1r, [BS, 1]); c_b1i = cl(b1i, [BS, 1])
            c_w2a = cl(w2a, [BS + 1, BS], BF16); c_w2b = cl(w2b, [BS + 1, BS], BF16)
            c_w2c = cl(w2c, [BS + 1, BS], BF16); c_w2d = cl(w2d, [BS + 1, BS], BF16)
            c_g1col = cl(g1col, [BS, 1]); c_spike = cl(spike, [BS, 1])
            c_b2rr = cl(b2rr, [1, BS], BF16); c_b2ir = cl(b2ir, [1, BS], BF16)
            c_ones1b = cl(ones1b, [1, 128], BF16)
            c_ones1 = cl(ones1, [1, 128]); c_ones128 = cl(ones128, [128, 1])
            c_ones128b = cl(ones128b, [128, 1], BF16)
            c_gbias = cl(gbias, [128, NMO]); c_fc2b = cl(fc2b, [128, NCC])
            c_g1f = cl(g1f, [128, NCC]); c_be1f = cl(be1f, [128, NCC])
            c_mask = cl(mask128, [NCORES, 128])
            c_eps = cp.tile([128, 1], F32, name="c_eps")
            nc.vector.memset(c_eps[:], EPS)

            qws = [QB[q + 1] - QB[q] for q in range(4)]
            a2a_in = [dram.tile([NCORES, BS, qws[q]], BF16, name=f"a2ai{q}")
                      for q in range(4)]
            a2a_out = [dram.tile([NCORES, BS, qws[q]], BF16, name=f"a2ao{q}")
                       for q in range(4)]
            st_in = [dram.tile([2, W, H], F32, name=f"st_in{b_}") for b_ in range(B)]
            st_out = [dram.tile([2, W, H], F32, name=f"st_out{b_}") for b_ in range(B)]

            # ================= phase 1 =================
            with ExitStack() as _st1:
                stp = _st1.enter_context(tc.tile_pool(name="stats", bufs=1))
                zp = _st1.enter_context(tc.tile_pool(name="zp", bufs=1))
                lnt = _st1.enter_context(tc.tile_pool(name="lnt", bufs=2))
                spA = _st1.enter_context(tc.tile_pool(name="spA", bufs=1))
                spB = _st1.enter_context(tc.tile_pool(name="spB", bufs=1))
                spQ = _st1.enter_context(tc.tile_pool(name="spQ", bufs=1))
                sbg = _st1.enter_context(tc.tile_pool(name="sbg", bufs=1))
                clp = _st1.enter_context(tc.tile_pool(name="clipp", bufs=2))
                pp = _st1.enter_context(tc.tile_pool(name="psum1", bufs=8, space="PSUM"))

                s_sum = [stp.tile([90, 2, H], F32, tag=f"ss{b_}", name=f"ssum{b_}") for b_ in range(B)]
                s_sq = [stp.tile([90, 2, H], F32, tag=f"sq{b_}", name=f"ssq{b_}") for b_ in range(B)]
                s_m = [stp.tile([90, 2, H], F32, tag=f"sm{b_}", name=f"sm{b_}") for b_ in range(B)]
                s_r = [stp.tile([90, 2, H], F32, tag=f"sr{b_}", name=f"sr{b_}") for b_ in range(B)]
                s_mr = [stp.tile([90, 2, H], BF16, tag=f"smr{b_}", name=f"smr{b_}") for b_ in range(B)]
                s_t = stp.tile([90, H], F32, tag="st_tmp", name="s_tmp")

                # resident bf16 x tiles: zhs[(b, wc, ch)] = [90, H, 48]
                zhs = {}
                for b in range(B):
                    for wc in range(2):
                        for ch in range(2):
                            zh = zp.tile([90, H, 48], BF16, tag=f"z{b}{wc}{ch}",
                                         name=f"zh{b}{wc}{ch}")
                            nc.sync.dma_start(zh[:], xw[wc * 90:(wc + 1) * 90, b, ch])
                            zhs[(b, wc, ch)] = zh

                # ---- stats pass (both b) + AllReduce per b
                def _stk(t, kind):
                    return bass.AP(tensor=t[:].tensor,
                                   offset=t[:].offset + kind * W * H,
                                   ap=[[90, 90], [8100, 2], [1, 90]])

                def gp_reduce48(dst, src, eng):
                    # sum src [90, H, 48] over last axis into dst [90, H],
                    # as a halving tree on the given engine
                    tA = lnt.tile([90, H, 24], F32, tag="rA", name="rA")
                    eng.tensor_add(tA[:], src[:, :, 0:24], src[:, :, 24:48])
                    tB = lnt.tile([90, H, 12], F32, tag="rB", name="rB")
                    eng.tensor_add(tB[:], tA[:, :, 0:12], tA[:, :, 12:24])
                    tC = lnt.tile([90, H, 6], F32, tag="rC", name="rC")
                    eng.tensor_add(tC[:], tB[:, :, 0:6], tB[:, :, 6:12])
                    tD = lnt.tile([90, H, 3], F32, tag="rD", name="rD")
                    eng.tensor_add(tD[:], tC[:, :, 0:3], tC[:, :, 3:6])
                    eng.tensor_add(dst, tD[:, :, 0], tD[:, :, 1])
                    eng.tensor_add(dst, dst, tD[:, :, 2])

                for b in range(B):
                    for wc in range(2):
                        for ch in range(2):
                            zh = zhs[(b, wc, ch)]
                            sqh = spQ.tile([90, H, 48], F32, tag="QU", name="sqh")
                            nc.scalar.activation(out=sqh[:], in_=zh[:], func=AF.Square)
                            teng = nc.gpsimd if b == 0 else nc.vector
                            if ch == 0:
                                nc.vector.reduce_sum(s_sum[b][:, wc, :], zh[:], axis=AX.X)
                                gp_reduce48(s_sq[b][:, wc, :], sqh, teng)
                            else:
                                nc.vector.reduce_sum(s_t[:], zh[:], axis=AX.X)
                                nc.vector.tensor_add(s_sum[b][:, wc, :], s_sum[b][:, wc, :], s_t[:])
                                sq2 = stp.tile([90, H], F32, tag="st_tmp2", name="s_tmp2")
                                gp_reduce48(sq2[:], sqh, teng)
                                teng.tensor_add(s_sq[b][:, wc, :], s_sq[b][:, wc, :], sq2[:])
                    nc.sync.dma_start(_stk(st_in[b], 0), s_sum[b][:])
                    nc.sync.dma_start(_stk(st_in[b], 1), s_sq[b][:])
                    nc.gpsimd.collective_compute(
                        "AllReduce", OP.add, replica_groups=rg,
                        ins=[st_in[b][:].opt()], outs=[st_out[b][:].opt()])

                for b in range(B):
                    nc.sync.dma_start(s_sum[b][:], _stk(st_out[b], 0))
                    nc.sync.dma_start(s_sq[b][:], _stk(st_out[b], 1))
                    nc.vector.tensor_scalar(out=s_m[b][:], in0=s_sum[b][:],
                                            scalar1=1.0 / C, scalar2=None,
                                            op0=OP.mult)
                    nc.vector.tensor_scalar(out=s_r[b][:], in0=s_sq[b][:],
                                            scalar1=1.0 / C, scalar2=None,
                                            op0=OP.mult)
                    tmp = stp.tile([90, 2, H], F32, tag=f"tmp{b}", name=f"tmpb{b}")
                    nc.vector.tensor_mul(tmp[:], s_m[b][:], s_m[b][:])
                    nc.vector.tensor_sub(s_r[b][:], s_r[b][:], tmp[:])
                    nc.scalar.activation(out=s_r[b][:], in_=s_r[b][:],
                                         func=AF.Sqrt, bias=c_eps[:90])
                    nc.vector.reciprocal(s_r[b][:], s_r[b][:])
                    # m*r rows for the rank-1 DFT correction of the LN mean
                    nc.vector.tensor_mul(s_mr[b][:], s_m[b][:], s_r[b][:])

                def _bcast48(t, b, wc):
                    # [90, H] stat slice broadcast along a trailing 48-axis
                    base = t[b][:]
                    return bass.AP(tensor=base.tensor,
                                   offset=base.offset + wc * H,
                                   ap=[[base.ap[0][0], 90], [1, H], [0, 48]])

                for b in range(B):
                    # ---- LN1 scale-only (z *= r, in place); the mean term
                    # is a channel-independent rank-1 correction applied at
                    # the F1 drain: yb = F1(z*r) - DFT_w(m*r).
                    yb = spA.tile([90, 92, BS], BF16, tag="YO", name="yb")
                    mrd = lnt.tile([90, 92], F32, tag="mrd", name="mrd")
                    pmr = pp.tile([90, 92], F32, tag="pp", name="psmr")
                    nc.tensor.matmul(pmr[:], s_mr[b][:, 0, :], c_fwp0[:],
                                     start=True, stop=False)
                    nc.tensor.matmul(pmr[:], s_mr[b][:, 1, :], c_fwp1[:],
                                     start=False, stop=True)
                    nc.vector.tensor_copy(mrd[:], pmr[:])
                    for ch in range(2):
                        for wc in range(2):
                            zt = zhs[(b, wc, ch)]
                            nc.gpsimd.tensor_mul(zt[:], zt[:],
                                                 _bcast48(s_r, b, wc))
                        zh0 = zhs[(b, 0, ch)]
                        zh1 = zhs[(b, 1, ch)]
                        # 5 output columns packed per PSUM bank -> 5x fewer
                        # (and 5x bigger) drains, fused with the -m*r fixup
                        for gi, g0 in enumerate(range(0, 48, 5)):
                            ncol = min(5, 48 - g0)
                            ps = pp.tile([90, 460], F32, tag="pp", name="psf1")
                            for idx in range(ncol):
                                cl_ = g0 + idx
                                sl = ps[:, idx * 92:(idx + 1) * 92]
                                nc.tensor.matmul(sl, zh0[:, :, cl_], c_fwp0[:],
                                                 start=True, stop=False)
                                nc.tensor.matmul(sl, zh1[:, :, cl_], c_fwp1[:],
                                                 start=False, stop=True)
                            c0 = ch * 48 + g0
                            src = bass.AP(tensor=ps[:].tensor,
                                          offset=ps[:].offset,
                                          ap=[[ps[:].ap[0][0], 90],
                                              [1, 92], [92, ncol]])
                            mrdv = bass.AP(tensor=mrd[:].tensor,
                                           offset=mrd[:].offset,
                                           ap=[[mrd[:].ap[0][0], 90],
                                               [1, 92], [0, ncol]])
                            nc.vector.tensor_sub(yb[:, :, c0:c0 + ncol],
                                                 src, mrdv)

                    # ---- F2 (5 wf packed per PSUM bank)
                    zb = spB.tile([BS, 2, KW, H], BF16, tag="ZO", name="zbt")
                    for g0 in range(0, KW, 5):
                        nwf = min(5, KW - g0)
                        prg = pp.tile([BS, 450], F32, tag="pp", name="psf2r")
                        pig = pp.tile([BS, 450], F32, tag="pp", name="psf2i")
                        for idx in range(nwf):
                            wf = g0 + idx
                            yr = yb[:, wf, :]
                            yi = yb[:, 46 + wf, :]
                            slr = prg[:, idx * 90:(idx + 1) * 90]
                            nc.tensor.matmul(slr, yr, c_fhc[:], start=True, stop=False)
                            nc.tensor.matmul(slr, yi, c_fhs[:], start=False, stop=True)
                            sli = pig[:, idx * 90:(idx + 1) * 90]
                            nc.tensor.matmul(sli, yi, c_fhc[:], start=True, stop=False)
                            nc.tensor.matmul(sli, yr, c_fhsm[:], start=False, stop=True)
                        nc.scalar.activation(out=zb[:, 0, g0:g0 + nwf, :],
                                             in_=prg[:, :nwf * 90],
                                             func=AF.Copy, scale=c_g1col[:])
                        nc.scalar.activation(out=zb[:, 1, g0:g0 + nwf, :],
                                             in_=pig[:, :nwf * 90],
                                             func=AF.Copy, scale=c_g1col[:])
                    nc.vector.tensor_scalar(out=zb[:, 0, 0, 0:1],
                                            in0=zb[:, 0, 0, 0:1],
                                            scalar1=c_spike[:], scalar2=None,
                                            op0=OP.add)

                    # ---- block MLP layer 1
                    o1 = spA.tile([BS + 1, 2, NM], BF16, tag="YO", name="o1t")
                    zr_f = zb[:, 0].rearrange("p a b -> p (a b)")
                    zi_f = zb[:, 1].rearrange("p a b -> p (a b)")
                    n0 = 0
                    while n0 < NM:
                        nn_ = min(512, NM - n0)
                        zr_s = zr_f[:, n0:n0 + nn_]
                        zi_s = zi_f[:, n0:n0 + nn_]
                        por = pp.tile([BS, 512], F32, tag="pp", name="pso1r")
                        nc.tensor.matmul(por[:, :nn_], c_w1r[:], zr_s,
                                         start=True, stop=False)
                        nc.tensor.matmul(por[:, :nn_], c_w1im[:], zi_s,
                                         start=False, stop=True)
                        poi = pp.tile([BS, 512], F32, tag="pp", name="pso1i")
                        nc.tensor.matmul(poi[:, :nn_], c_w1i[:], zr_s,
                                         start=True, stop=False)
                        nc.tensor.matmul(poi[:, :nn_], c_w1r[:], zi_s,
                                         start=False, stop=True)
                        nc.scalar.activation(out=o1[0:BS, 0, n0:n0 + nn_],
                                             in_=por[:, :nn_], func=AF.Relu,
                                             bias=c_b1r[:])
                        nc.scalar.activation(out=o1[0:BS, 1, n0:n0 + nn_],
                                             in_=poi[:, :nn_], func=AF.Relu,
                                             bias=c_b1i[:])
                        n0 += nn_

                    # ---- block MLP layer 2 + softshrink (5 wf packed/bank)
                    o2 = spB.tile([H, 2, KW, BS], BF16, tag="ZO", name="o2t")
                    o1r_f = o1[:, 0]
                    o1i_f = o1[:, 1]
                    for g0 in range(0, KW, 5):
                        nwf = min(5, KW - g0)
                        prg = pp.tile([H, 480], F32, tag="pp", name="pso2r")
                        pig = pp.tile([H, 480], F32, tag="pp", name="pso2i")
                        for idx in range(nwf):
                            wf = g0 + idx
                            lr = o1r_f[0:BS, wf * H:(wf + 1) * H]
                            li = o1i_f[0:BS, wf * H:(wf + 1) * H]
                            slr = prg[:, idx * BS:(idx + 1) * BS]
                            nc.tensor.matmul(slr, lr, c_w2a[0:BS, :], start=True, stop=False)
                            nc.tensor.matmul(slr, li, c_w2b[0:BS, :], start=False, stop=False)
                            nc.tensor.matmul(slr, c_ones1b[:, 0:H], c_b2rr[:], start=False, stop=True)
                            sli = pig[:, idx * BS:(idx + 1) * BS]
                            nc.tensor.matmul(sli, li, c_w2c[0:BS, :], start=True, stop=False)
                            nc.tensor.matmul(sli, lr, c_w2d[0:BS, :], start=False, stop=False)
                            nc.tensor.matmul(sli, c_ones1b[:, 0:H], c_b2ir[:], start=False, stop=True)
                        for ri, psm in ((0, prg), (1, pig)):
                            clip = clp.tile([H, 480], F32, tag="clip", name="clipt")
                            nc.vector.tensor_scalar(out=clip[:, :nwf * BS],
                                                    in0=psm[:, :nwf * BS],
                                                    scalar1=-LAM, scalar2=LAM,
                                                    op0=OP.max, op1=OP.min)
                            nc.vector.tensor_sub(o2[:, ri, g0:g0 + nwf, :],
                                                 psm[:, :nwf * BS],
                                                 clip[:, :nwf * BS])

                    # ---- inverse H-DFT -> u2r/u2i [46, (c, h)] (5 c / bank)
                    u2r = spQ.tile([KW, BS, H], BF16, tag="QU", name="u2rt")
                    u2i = spA.tile([KW, BS, H], BF16, tag="YO", name="u2it")
                    for g0 in range(0, BS, 5):
                        ncl = min(5, BS - g0)
                        purg = pp.tile([KW, 450], F32, tag="pp", name="psur")
                        puig = pp.tile([KW, 450], F32, tag="pp", name="psui")
                        for idx in range(ncl):
                            c = g0 + idx
                            lr = o2[:, 0, :, c]
                            li = o2[:, 1, :, c]
                            slr = purg[:, idx * H:(idx + 1) * H]
                            nc.tensor.matmul(slr, lr, c_fhc[:], start=True, stop=False)
                            nc.tensor.matmul(slr, li, c_fhsm[:], start=False, stop=True)
                            sli = puig[:, idx * H:(idx + 1) * H]
                            nc.tensor.matmul(sli, li, c_fhc[:], start=True, stop=False)
                            nc.tensor.matmul(sli, lr, c_fhs[:], start=False, stop=True)
                        nc.scalar.activation(out=u2r[:, g0:g0 + ncl, :],
                                             in_=purg[:, :ncl * H], func=AF.Copy)
                        nc.vector.tensor_copy(u2i[:, g0:g0 + ncl, :],
                                              puig[:, :ncl * H])

                    # ---- inverse W-DFT -> SBUF gather sbA (bf16), 2 c/bank
                    sbA = sbg.tile([H, BS, W], BF16, tag="sbA", name="sbA")
                    for c0 in range(0, BS, 2):
                        pf = pp.tile([H, 360], F32, tag="pp", name="psf")
                        for idx in range(2):
                            c = c0 + idx
                            sl = pf[:, idx * W:(idx + 1) * W]
                            nc.tensor.matmul(sl, u2r[:, c, :], c_iwrt[:],
                                             start=True, stop=False)
                            nc.tensor.matmul(sl, u2i[:, c, :], c_iwit[:],
                                             start=False, stop=True)
                        if (c0 // 2) % 2 == 0:
                            nc.scalar.activation(out=sbA[:, c0:c0 + 2, :],
                                                 in_=pf[:], func=AF.Copy)
                        else:
                            nc.vector.tensor_copy(sbA[:, c0:c0 + 2, :], pf[:])

                    # ---- a2a send pieces (SBUF -> DRAM), quarter-major
                    # so quarter 0's collective can fire first
                    for q in range(4):
                        qw = qws[q]
                        for jq in range(4):
                            j = b * 4 + jq
                            for (ha, hb_, wa, wb_, toff) in _shard_pieces(
                                    jq, QB[q], QB[q + 1]):
                                src = sbA[ha:hb_, :, wa:wb_]
                                dst = bass.AP(
                                    tensor=a2a_in[q][:].tensor,
                                    offset=a2a_in[q][:].offset
                                    + (j * BS * qw + toff),
                                    ap=[[wb_ - wa, hb_ - ha], [qw, BS],
                                        [1, wb_ - wa]])
                                nc.sync.dma_start(dst, src)

            for q in range(4):
                nc.gpsimd.collective_compute(
                    "AllToAll", OP.bypass, replica_groups=rg,
                    ins=[a2a_in[q][:].opt()], outs=[a2a_out[q][:].opt()])

            # ================= phase 2 =================
            with ExitStack() as _st2:
                fc1p = _st2.enter_context(tc.tile_pool(name="fc1p", bufs=1))
                fc2p = _st2.enter_context(tc.tile_pool(name="fc2p", bufs=1))
                lnp = _st2.enter_context(tc.tile_pool(name="lnp", bufs=1))

                FP8 = mybir.dt.float8e4
                c_fc1 = [fc1p.tile([128, 2, HID], FP8, tag=f"fc1_{i}",
                                   name=f"cfc1_{i}") for i in range(3)]
                for i in range(3):
                    nc.sync.dma_start(c_fc1[i][:], fc1q[i])
                c_fc2 = [fc2p.tile([128, 2, C], FP8, tag=f"fc2_{i}",
                                   name=f"cfc2_{i}") for i in range(12)]
                for i in range(12):
                    nc.sync.dma_start(c_fc2[i][:], fc2q[i])

                R1B = lnp.tile([128, TSH], BF16, tag="R1B", name="R1B")
                MR1B = lnp.tile([128, TSH], BF16, tag="MR1B", name="MR1B")

                with ExitStack() as _stR:
                    rowp = _stR.enter_context(tc.tile_pool(name="rowp", bufs=1))
                    pbR = _stR.enter_context(
                        tc.tile_pool(name="pbR", bufs=2, space="PSUM"))
                    # all-shard LN1 stats rows [8, 4050], computed in place
                    r1_8 = rowp.tile([NCORES, TSH], F32, tag="r18", name="r18")
                    mr1_8 = rowp.tile([NCORES, TSH], F32, tag="mr18", name="mr18")
                    rtmp = rowp.tile([NCORES, TSH], F32, tag="rtmp", name="rtmp")
                    for kind, dstt in ((0, rtmp), (1, r1_8)):
                        for s in range(NCORES):
                            bb, jq = s // 4, s % 4
                            for (ha, hb_, wa, wb_, toff) in _shard_pieces(jq):
                                src_ = bass.AP(
                                    tensor=st_out[bb][:].tensor,
                                    offset=st_out[bb][:].offset
                                    + (kind * W * H + wa * H + ha),
                                    ap=[[0, 1], [1, hb_ - ha], [H, wb_ - wa]])
                                nc.sync.dma_start(
                                    dstt[s:s + 1,
                                         toff:toff + (hb_ - ha) * (wb_ - wa)],
                                    src_)
                    nc.vector.tensor_scalar(out=mr1_8[:], in0=rtmp[:],
                                            scalar1=1.0 / C, scalar2=None,
                                            op0=OP.mult)           # m1
                    nc.vector.tensor_scalar(out=r1_8[:], in0=r1_8[:],
                                            scalar1=1.0 / C, scalar2=None,
                                            op0=OP.mult)           # q/C
                    nc.vector.tensor_mul(rtmp[:], mr1_8[:], mr1_8[:])
                    nc.vector.tensor_sub(r1_8[:], r1_8[:], rtmp[:])  # var
                    nc.scalar.activation(out=r1_8[:], in_=r1_8[:], func=AF.Sqrt,
                                         bias=c_eps[:NCORES])
                    nc.vector.reciprocal(r1_8[:], r1_8[:])           # r1
                    nc.vector.tensor_mul(mr1_8[:], mr1_8[:], r1_8[:])  # m1*r1

                    # pre-broadcast this core's r1 / m1*r1 to bf16 planes
                    for it, T in enumerate(TTS):
                        t0 = TT0[it]
                        for rows, plane in ((r1_8, R1B), (mr1_8, MR1B)):
                            pb = pbR.tile([128, 512], F32, tag="pbc")
                            nc.tensor.matmul(pb[:, :T], c_mask[:],
                                             rows[:, t0:t0 + T],
                                             start=True, stop=True)
                            nc.scalar.activation(out=plane[:, t0:t0 + T],
                                                 in_=pb[:, :T], func=AF.Copy)

                xcp = _st2.enter_context(tc.tile_pool(name="xcp", bufs=6))
                rcvp = _st2.enter_context(tc.tile_pool(name="rcvp", bufs=6))
                t1p = _st2.enter_context(tc.tile_pool(name="t1p", bufs=3))
                htokp = _st2.enter_context(tc.tile_pool(name="htokp", bufs=12))
                htnp = _st2.enter_context(tc.tile_pool(name="htnp", bufs=12))
                hidp = _st2.enter_context(tc.tile_pool(name="hidp", bufs=1))
                rw2 = _st2.enter_context(tc.tile_pool(name="rw2", bufs=2))
                bcp = _st2.enter_context(tc.tile_pool(name="bcp", bufs=2))
                outp = _st2.enter_context(tc.tile_pool(name="outp", bufs=3))
                ph = _st2.enter_context(tc.tile_pool(name="ph", bufs=2, space="PSUM"))
                po = _st2.enter_context(tc.tile_pool(name="po", bufs=2, space="PSUM"))
                pst = _st2.enter_context(tc.tile_pool(name="pst", bufs=2, space="PSUM"))
                pbc = _st2.enter_context(tc.tile_pool(name="pbc", bufs=2, space="PSUM"))

                # software-pipelined chunk loop: front half (loads, assemble,
                # LN2 stats, normalize) runs one chunk ahead of the back half
                # (fc1 -> gelu -> fc2 -> residual -> store).
                hts = {}
                htns = {}
                hid = hidp.tile([128, NMO, 512], FP8, tag="hid", name="hid")
                for i in range(NCH + 1):
                    if i < NCH:
                        T = TTS[i]
                        t0 = TT0[i]
                        htoks = []
                        htnl = []
                        for cc in range(NCC):
                            xct = xcp.tile([128, 512], F32, tag="xct")
                            nc.sync.dma_start(xct[:, :T],
                                              xc[cc * 128:(cc + 1) * 128, t0:t0 + T])
                            rcv = rcvp.tile([128, 512], BF16, tag="rcv")
                            q = QMAP[i]
                            lt0 = t0 - QB[q]
                            c0 = cc * 128
                            r0 = 0
                            while r0 < 128:
                                s_blk = (c0 + r0) // BS
                                c_in = (c0 + r0) % BS
                                nrow = min(BS - c_in, 128 - r0)
                                nc.sync.dma_start(
                                    rcv[r0:r0 + nrow, :T],
                                    a2a_out[q][s_blk, c_in:c_in + nrow,
                                               lt0:lt0 + T])
                                r0 += nrow
                            ht = htokp.tile([128, 512], F32, tag="htok")
                            htoks.append(ht)
                            t1 = t1p.tile([128, 512], F32, tag="t1")
                            nc.gpsimd.tensor_mul(t1[:, :T], xct[:, :T],
                                                 R1B[:, t0:t0 + T])
                            nc.vector.tensor_sub(t1[:, :T], t1[:, :T],
                                                 MR1B[:, t0:t0 + T])
                            nc.vector.tensor_scalar(out=t1[:, :T], in0=t1[:, :T],
                                                    scalar1=c_g1f[:, cc:cc + 1],
                                                    scalar2=c_be1f[:, cc:cc + 1],
                                                    op0=OP.mult, op1=OP.add)
                            nc.gpsimd.tensor_add(ht[:, :T], rcv[:, :T], xct[:, :T])
                            nc.vector.tensor_add(ht[:, :T], ht[:, :T], t1[:, :T])

                        # LN2 stats via bf16 ones-matmul (1 cyc/row vs 4)
                        ps_s = pst.tile([1, 512], F32, tag="pst")
                        ps_q = pst.tile([1, 512], F32, tag="pst")
                        htbs = []
                        for cc in range(NCC):
                            htb = t1p.tile([128, 512], BF16, tag="htb", name="htb")
                            nc.scalar.activation(out=htb[:, :T],
                                                 in_=htoks[cc][:, :T],
                                                 func=AF.Copy)
                            htbs.append(htb)
                        for cc in range(NCC):
                            nc.tensor.matmul(ps_s[:, :T], c_ones128b[:],
                                             htbs[cc][:, :T],
                                             start=(cc == 0), stop=(cc == NCC - 1))
                        hsqs = []
                        for cc in range(NCC):
                            hsq = t1p.tile([128, 512], BF16, tag="hsq", name="hsq")
                            nc.gpsimd.tensor_mul(hsq[:, :T], htbs[cc][:, :T],
                                                 htbs[cc][:, :T])
                            hsqs.append(hsq)
                        for cc in range(NCC):
                            nc.tensor.matmul(ps_q[:, :T], c_ones128b[:],
                                             hsqs[cc][:, :T],
                                             start=(cc == 0), stop=(cc == NCC - 1))
                        m2r = rw2.tile([1, 512], F32, tag="m2r")
                        r2r = rw2.tile([1, 512], F32, tag="r2r")
                        vv = rw2.tile([1, 512], F32, tag="vv")
                        nc.vector.tensor_scalar(out=m2r[:, :T], in0=ps_s[:, :T],
                                                scalar1=1.0 / C, scalar2=None,
                                                op0=OP.mult)
                        nc.vector.tensor_scalar(out=r2r[:, :T], in0=ps_q[:, :T],
                                                scalar1=1.0 / C, scalar2=None,
                                                op0=OP.mult)
                        nc.vector.tensor_mul(vv[:, :T], m2r[:, :T], m2r[:, :T])
                        nc.vector.tensor_sub(r2r[:, :T], r2r[:, :T], vv[:, :T])
                        nc.scalar.activation(out=r2r[:, :T], in_=r2r[:, :T],
                                             func=AF.Sqrt, bias=c_eps[:1])
                        nc.vector.reciprocal(r2r[:, :T], r2r[:, :T])
                        # broadcast m2, r2 to all partitions
                        m2b = bcp.tile([128, 512], F32, tag="m2b")
                        r2b = bcp.tile([128, 512], F32, tag="r2b")
                        for rowt, bt in ((m2r, m2b), (r2r, r2b)):
                            pb = pbc.tile([128, 512], F32, tag="pbc")
                            nc.tensor.matmul(pb[:, :T], c_ones1[:], rowt[:, :T],
                                             start=True, stop=True)
                            nc.scalar.activation(out=bt[:, :T], in_=pb[:, :T],
                                                 func=AF.Copy)
                        # normalize -> fp8e4m3 (DoubleRow slot layout)
                        for cc in range(NCC):
                            if cc % 2 == 0:
                                htn = htnp.tile([128, 2, 512], FP8, tag="htn")
                                htnl.append(htn)
                            tn = t1p.tile([128, 512], F32, tag="t1", name="tn")
                            nc.vector.tensor_sub(tn[:, :T], htoks[cc][:, :T],
                                                 m2b[:, :T])
                            nc.vector.tensor_mul(htn[:, cc % 2, :T], tn[:, :T],
                                                 r2b[:, :T])
                        hts[i] = htoks
                        htns[i] = htnl

                    if i >= 1:
                        j = i - 1
                        T = TTS[j]
                        t0 = TT0[j]
                        htoks = hts.pop(j)
                        htnl = htns.pop(j)
                        # fc1 (fp8 DoubleRow) + gelu -> hid (fp8)
                        for mo in range(NMO):
                            php = ph.tile([128, 512], F32, tag="ph")
                            for p_ in range(3):
                                nc.tensor.matmul(
                                    php[:, :T],
                                    c_fc1[p_][:, :, mo * 128:(mo + 1) * 128],
                                    htnl[p_][:, :, :T],
                                    start=(p_ == 0), stop=(p_ == 2),
                                    perf_mode=mybir.MatmulPerfMode.DoubleRow)
                            nc.scalar.activation(out=hid[:, mo, :T],
                                                 in_=php[:, :T], func=AF.Gelu,
                                                 scale=1.0 / WSCALE,
                                                 bias=c_gbias[:, mo:mo + 1])
                        # fc2 (fp8 DoubleRow) + bias + residual -> store
                        for co in range(NCC):
                            pop = po.tile([128, 512], F32, tag="po")
                            for p_ in range(12):
                                nc.tensor.matmul(
                                    pop[:, :T],
                                    c_fc2[p_][:, :, co * 128:(co + 1) * 128],
                                    hid[:, 2 * p_:2 * p_ + 2, :T],
                                    start=(p_ == 0), stop=(p_ == 11),
                                    perf_mode=mybir.MatmulPerfMode.DoubleRow)
                            osb = outp.tile([128, 512], F32, tag="osb")
                            nc.scalar.activation(out=osb[:, :T],
                                                 in_=pop[:, :T],
                                                 func=AF.Identity,
                                                 scale=1.0 / WSCALE,
                                                 bias=c_fc2b[:, co:co + 1])
                            nc.vector.tensor_add(osb[:, :T], osb[:, :T],
                                                 htoks[co][:, :T])
                            nc.sync.dma_start(
                                out[co * 128:(co + 1) * 128, t0:t0 + T],
                                osb[:, :T])

    nc.compile()
    _CACHE["nc"] = nc
    return nc


def _host_prep(inputs):
    x = np.ascontiguousarray(np.asarray(inputs["x"], dtype=np.float32))
    g1 = np.asarray(inputs["g1"], np.float32); be1 = np.asarray(inputs["be1"], np.float32)
    g2 = np.asarray(inputs["g2"], np.float32); be2 = np.asarray(inputs["be2"], np.float32)
    w1 = np.asarray(inputs["w1"], np.float32); b1 = np.asarray(inputs["b1"], np.float32)
    w2 = np.asarray(inputs["w2"], np.float32); b2 = np.asarray(inputs["b2"], np.float32)
    fc1_w = np.asarray(inputs["fc1_w"], np.float32)
    fc1_b = np.asarray(inputs["fc1_b"], np.float32)
    fc2_w = np.asarray(inputs["fc2_w"], np.float32)
    fc2_b = np.asarray(inputs["fc2_b"], np.float32)

    fwp0, fwp1, fhc_m, fhs_m, fhsm_m, iwrt_m, iwit_m = _dft_consts()
    xf = x.reshape(TOK, C)
    bf = ml_dtypes.bfloat16
    f8 = ml_dtypes.float8_e4m3fn
    fc1q_m = np.ascontiguousarray(
        (g2[:, None] * fc1_w * WSCALE).reshape(3, 2, 128, HID)
        .transpose(0, 2, 1, 3)).astype(f8)                     # (3,128,2,3072)
    fc2q_m = np.ascontiguousarray(
        (fc2_w * WSCALE).reshape(12, 2, 128, C)
        .transpose(0, 2, 1, 3)).astype(f8)                     # (12,128,2,768)
    gbias_v = (fc1_b + be2 @ fc1_w).astype(np.float32)         # (3072,)
    gbias_m = np.ascontiguousarray(gbias_v.reshape(NMO, 128).T)  # (128, 24)
    fc2b_m = np.ascontiguousarray(fc2_b.reshape(NCC, 128).T)
    g1f_m = np.ascontiguousarray(g1.reshape(NCC, 128).T)
    be1f_m = np.ascontiguousarray(be1.reshape(NCC, 128).T)
    ones1 = np.ones((1, 128), np.float32)
    ones1b = np.ones((1, 128), bf)
    ones128 = np.ones((128, 1), np.float32)
    ones128b = np.ones((128, 1), bf)

    in_maps = []
    for k in range(NCORES):
        ck = slice(k * BS, (k + 1) * BS)
        # (W, B, H, 96) -> (W, B, 2, H, 48), bf16
        xw_k = np.ascontiguousarray(
            x[:, :, :, ck].transpose(2, 0, 1, 3)
            .reshape(W, B, H, 2, 48).transpose(0, 1, 3, 2, 4)).astype(bf)
        xc_k = np.ascontiguousarray(xf[k * TSH:(k + 1) * TSH, :].T)
        w1r_k = np.ascontiguousarray(w1[k, :, :, 0])
        w1i_k = np.ascontiguousarray(w1[k, :, :, 1])
        w2r_k = np.ascontiguousarray(w2[k, :, :, 0])
        w2i_k = np.ascontiguousarray(w2[k, :, :, 1])
        b2r_k = b2[k, :, 0]; b2i_k = b2[k, :, 1]
        zr = np.zeros((1, BS), np.float32)
        mask = np.zeros((NCORES, 128), np.float32); mask[k, :] = 1.0
        in_maps.append({
            "xw": xw_k, "xc": xc_k,
            "fwp0": fwp0.astype(bf), "fwp1": fwp1.astype(bf),
            "fhc": fhc_m.astype(bf), "fhs": fhs_m.astype(bf),
            "fhsm": fhsm_m.astype(bf),
            "iwrt": iwrt_m.astype(bf), "iwit": iwit_m.astype(bf),
            "w1r": w1r_k.astype(bf), "w1i": w1i_k.astype(bf),
            "w1im": (-w1i_k).astype(bf),
            "b1r": b1[k, :, 0:1].copy(), "b1i": b1[k, :, 1:2].copy(),
            "w2a": np.concatenate([w2r_k, b2r_k[None, :]], 0).astype(bf),
            "w2b": np.concatenate([-w2i_k, zr], 0).astype(bf),
            "w2c": np.concatenate([w2r_k, zr], 0).astype(bf),
            "w2d": np.concatenate([w2i_k, b2i_k[None, :]], 0).astype(bf),
            "g1col": g1[ck][:, None].copy(),
            "b2rr": b2r_k[None, :].astype(bf), "b2ir": b2i_k[None, :].astype(bf),
            "spike": (be1[ck] * SQN)[:, None].astype(np.float32),
            "fc1q": fc1q_m, "fc2q": fc2q_m, "gbias": gbias_m,
            "fc2b": fc2b_m, "g1f": g1f_m, "be1f": be1f_m,
            "ones1": ones1, "ones1b": ones1b, "ones128": ones128,
            "ones128b": ones128b,
            "mask128": mask,
        })
    return in_maps


def kernel(**inputs):
    nc = _build_nc()
    in_maps = _host_prep(inputs)
    res = run_bass_kernel_spmd(nc, in_maps, core_ids=list(range(NCORES)))
    outs = [np.asarray(res.results[j]["out"], dtype=np.float32).T
            for j in range(NCORES)]
    full = np.concatenate(outs, axis=0).reshape(B, H, W, C)
    return np.ascontiguousarray(full, dtype=np.float32)


# revision 36
# speedup vs baseline: 1.0568x; 1.0480x over previous
"""AFNO transformer block on 8 Trainium2 NeuronCores.

Distribution:
  Phase 1 (channel-block sharded): core k owns channels [96k, 96k+96).
    LN1 stats partial sums -> per-batch AllReduce -> LN1 apply, then the
    whole spectral path (rFFT2 as DFT matmuls, block-diagonal complex MLP,
    inverse rFFT2) entirely core-local. DFTs use "flip" matmuls (data as
    the stationary operand) so every stage lands in the layout the next
    stage contracts over - no on-chip transposes.
  AllToAll (bf16 payload): filter output reshard (channel -> token).
  Phase 2 (token sharded): core j owns tokens [4050j, 4050j+4050).
    h = filt + LN1(x) + x assembled channel-major; LN1 per-token scales
    pre-broadcast to bf16 [128, 4050] planes via mask matmuls. LN2 stats
    by ones-matmul, normalize-first (htn = (h-m2)*r2 in bf16), then
    bf16 fc1 (g2-folded) -> exact Gelu -> bf16 fc2 -> residual. Weights
    for fc1 AND fc2 stay resident in SBUF in bf16. Output is written
    channel-major [C, TSH]; the host transposes.
"""
import math
import numpy as np
import ml_dtypes

import concourse.bass as bass
import concourse.mybir as mybir
import concourse.tile as tile
from concourse import bacc
from concourse.bass_utils import run_bass_kernel_spmd

F32 = mybir.dt.float32
F32R = mybir.dt.float32r
BF16 = mybir.dt.bfloat16
AF = mybir.ActivationFunctionType
OP = mybir.AluOpType
AX = mybir.AxisListType

NCORES = 8
B, H, W, C = 2, 90, 180, 768
BS = 96           # channels per core / AFNO block size
KW = 46           # kept W-frequency modes
HID = 3072
LAM = 0.01
EPS = 1e-5
TOK = B * H * W   # 32400
TSH = TOK // NCORES  # 4050
NM = KW * H       # modes per batch elem: 4140
SQN = math.sqrt(H * W)

QB = [0, 506, 1519, 2532, 4050]      # a2a quarter boundaries (tokens)
TTS = [506, 507, 506, 507, 506, 506, 506, 506]
QMAP = [0, 1, 1, 2, 2, 3, 3, 3]      # chunk -> a2a quarter
TT0 = [sum(TTS[:i]) for i in range(len(TTS))]
NCH = len(TTS)
NCC = 6    # 768/128
WSCALE = 64.0  # fp8 weight pre-scale for fc1/fc2
NMO = 24   # 3072/128


def _dft_consts():
    wv = np.arange(W, dtype=np.float64)[:, None]
    wf = np.arange(KW, dtype=np.float64)[None, :]
    ang = 2.0 * np.pi * wv * wf / W
    fwr = np.cos(ang) / math.sqrt(W)
    fwi = -np.sin(ang) / math.sqrt(W)
    fwpack = np.concatenate([fwr, fwi], axis=1)          # (180, 92)
    hv = np.arange(H, dtype=np.float64)[:, None]
    hf = np.arange(H, dtype=np.float64)[None, :]
    angh = 2.0 * np.pi * hv * hf / H
    fhc = np.cos(angh) / math.sqrt(H)                    # symmetric
    fhs = np.sin(angh) / math.sqrt(H)
    alpha = np.ones(KW); alpha[1:] = 2.0
    iwr = alpha[None, :] * np.cos(ang) / math.sqrt(W)    # (180, 46)
    iwi = -alpha[None, :] * np.sin(ang) / math.sqrt(W)
    f32 = np.float32
    return (fwpack[:90].astype(f32), fwpack[90:].astype(f32),
            fhc.astype(f32), fhs.astype(f32), (-fhs).astype(f32),
            np.ascontiguousarray(iwr.T).astype(f32),
            np.ascontiguousarray(iwi.T).astype(f32))


def _shard_pieces(jq, lo=0, hi=TSH):
    """(ha,hb,wa,wb,tok_off) pieces of within-batch shard jq, tokens
    [lo, hi) of the shard; tok_off is relative to lo."""
    s0, e0 = TSH * jq + lo, TSH * jq + hi
    pieces, t = [], s0
    while t < e0:
        h = t // W
        wa = t - h * W
        if wa != 0 or e0 - t < W:
            wb = min(W, wa + (e0 - t))
            pieces.append((h, h + 1, wa, wb, t - s0))
            t += wb - wa
        else:
            hb = min(H, h + (e0 - t) // W)
            pieces.append((h, hb, 0, W, t - s0))
            t += (hb - h) * W
    return pieces


_CACHE = {}


def _build_nc():
    if "nc" in _CACHE:
        return _CACHE["nc"]
    nc = bacc.Bacc("TRN2", target_bir_lowering=False, debug=False,
                   num_devices=NCORES)
    g = lambda n, s, dt=F32: nc.dram_tensor(n, s, dt, kind="ExternalInput")
    xw = g("xw", [W, B, 2, H, 48], BF16)
    xc = g("xc", [C, TSH])
    fwp0 = g("fwp0", [90, 92], BF16); fwp1 = g("fwp1", [90, 92], BF16)
    fhc = g("fhc", [90, 90], BF16); fhs = g("fhs", [90, 90], BF16)
    fhsm = g("fhsm", [90, 90], BF16)
    iwrt = g("iwrt", [KW, W], BF16); iwit = g("iwit", [KW, W], BF16)
    w1r = g("w1r", [BS, BS], BF16); w1i = g("w1i", [BS, BS], BF16)
    w1im = g("w1im", [BS, BS], BF16)
    b1r = g("b1r", [BS, 1]); b1i = g("b1i", [BS, 1])
    w2a = g("w2a", [BS + 1, BS], BF16); w2b = g("w2b", [BS + 1, BS], BF16)
    w2c = g("w2c", [BS + 1, BS], BF16); w2d = g("w2d", [BS + 1, BS], BF16)
    g1col = g("g1col", [BS, 1]); spike = g("spike", [BS, 1])
    b2rr = g("b2rr", [1, BS], BF16); b2ir = g("b2ir", [1, BS], BF16)
    ones1b = g("ones1b", [1, 128], BF16)
    FP8 = mybir.dt.float8e4
    fc1q = g("fc1q", [3, 128, 2, HID], FP8)
    fc2q = g("fc2q", [12, 128, 2, C], FP8)
    gbias = g("gbias", [128, NMO])
    fc2b = g("fc2b", [128, NCC])
    g1f = g("g1f", [128, NCC]); be1f = g("be1f", [128, NCC])
    ones1 = g("ones1", [1, 128])
    ones128 = g("ones128", [128, 1])
    ones128b = g("ones128b", [128, 1], BF16)
    mask128 = g("mask128", [NCORES, 128])   # one-hot row = this core's shard

    out = nc.dram_tensor("out", [C, TSH], F32, kind="ExternalOutput")
    rg = [list(range(NCORES))]

    from contextlib import ExitStack
    with tile.TileContext(nc) as tc:
        with ExitStack() as _st0:
            cp = _st0.enter_context(tc.tile_pool(name="const", bufs=1))
            dram = _st0.enter_context(tc.tile_pool(name="dram", bufs=1, space="DRAM"))
            def cl(t, shape, dt=F32):
                nm = f"c_{t.name if hasattr(t, 'name') else t[:].tensor.name}"
                s = cp.tile(shape, dt, name=nm, tag=nm)
                nc.scalar.dma_start(s[:], t[:])
                return s
            c_fwp0 = cl(fwp0, [90, 92], BF16); c_fwp1 = cl(fwp1, [90, 92], BF16)
            c_fhc = cl(fhc, [90, 90], BF16); c_fhs = cl(fhs, [90, 90], BF16)
            c_fhsm = cl(fhsm, [90, 90], BF16)
            c_iwrt = cl(iwrt, [KW, W], BF16); c_iwit = cl(iwit, [KW, W], BF16)
            c_w1r = cl(w1r, [BS, BS], BF16); c_w1i = cl(w1i, [BS, BS], BF16)
            c_w1im = cl(w1im, [BS, BS], BF16)
            c_b1r = cl(b1r, [BS, 1]); c_b1i = cl(b1i, [BS, 1])
            c_w2a = cl(w2a, [BS + 1, BS], BF16); c_w2b = cl(w2b, [BS + 1, BS], BF16)
            c_w2c = cl(w2c, [BS + 1, BS], BF16); c_w2d = cl(w2d, [BS + 1, BS], BF16)
            c_g1col = cl(g1col, [BS, 1]); c_spike = cl(spike, [BS, 1])
            c_b2rr = cl(b2rr, [1, BS], BF16); c_b2ir = cl(b2ir, [1, BS], BF16)
            c_ones1b = cl(ones1b, [1, 128], BF16)
            c_ones1 = cl(ones1, [1, 128]); c_ones128 = cl(ones128, [128, 1])
            c_ones128b = cl(ones128b, [128, 1], BF16)
            c_gbias = cl(gbias, [128, NMO]); c_fc2b = cl(fc2b, [128, NCC])
            c_g1f = cl(g1f, [128, NCC]); c_be1f = cl(be1f, [128, NCC])
            c_mask = cl(mask128, [NCORES, 128])
            c_eps = cp.tile([128, 1], F32, name="c_eps")
            nc.vector.memset(c_eps[:], EPS)

            qws = [QB[q + 1] - QB[q] for q in range(4)]
            a2a_in = [dram.tile([NCORES, BS, qws[q]], BF16, name=f"a2ai{q}")
                      for q in range(4)]
            a2a_out = [dram.tile([NCORES, BS, qws[q]], BF16, name=f"a2ao{q}")
                       for q in range(4)]
            st_in = [dram.tile([2, W, H], F32, name=f"st_in{b_}") for b_ in range(B)]
            st_out = [dram.tile([2, W, H], F32, name=f"st_out{b_}") for b_ in range(B)]

            # ================= phase 1 =================
            with ExitStack() as _st1:
                stp = _st1.enter_context(tc.tile_pool(name="stats", bufs=1))
                zp = _st1.enter_context(tc.tile_pool(name="zp", bufs=1))
                lnt = _st1.enter_context(tc.tile_pool(name="lnt", bufs=2))
                spA = _st1.enter_context(tc.tile_pool(name="spA", bufs=1))
                spB = _st1.enter_context(tc.tile_pool(name="spB", bufs=1))
                spQ = _st1.enter_context(tc.tile_pool(name="spQ", bufs=1))
                sbg = _st1.enter_context(tc.tile_pool(name="sbg", bufs=1))
                clp = _st1.enter_context(tc.tile_pool(name="clipp", bufs=2))
                pp = _st1.enter_context(tc.tile_pool(name="psum1", bufs=8, space="PSUM"))

                s_sum = [stp.tile([90, 2, H], F32, tag=f"ss{b_}", name=f"ssum{b_}") for b_ in range(B)]
                s_sq = [stp.tile([90, 2, H], F32, tag=f"sq{b_}", name=f"ssq{b_}") for b_ in range(B)]
                s_m = [stp.tile([90, 2, H], F32, tag=f"sm{b_}", name=f"sm{b_}") for b_ in range(B)]
                s_r = [stp.tile([90, 2, H], F32, tag=f"sr{b_}", name=f"sr{b_}") for b_ in range(B)]
                s_mr = [stp.tile([90, 2, H], BF16, tag=f"smr{b_}", name=f"smr{b_}") for b_ in range(B)]
                s_t = stp.tile([90, H], F32, tag="st_tmp", name="s_tmp")

                # resident bf16 x tiles: zhs[(b, wc, ch)] = [90, H, 48]
                zhs = {}
                for b in range(B):
                    for wc in range(2):
                        for ch in range(2):
                            zh = zp.tile([90, H, 48], BF16, tag=f"z{b}{wc}{ch}",
                                         name=f"zh{b}{wc}{ch}")
                            nc.sync.dma_start(zh[:], xw[wc * 90:(wc + 1) * 90, b, ch])
                            zhs[(b, wc, ch)] = zh

                # ---- stats pass (both b) + AllReduce per b
                def _stk(t, kind):
                    return bass.AP(tensor=t[:].tensor,
                                   offset=t[:].offset + kind * W * H,
                                   ap=[[90, 90], [8100, 2], [1, 90]])

                def gp_reduce48(dst, src, eng):
                    # sum src [90, H, 48] over last axis into dst [90, H],
                    # as a halving tree on the given engine
                    tA = lnt.tile([90, H, 24], F32, tag="rA", name="rA")
                    eng.tensor_add(tA[:], src[:, :, 0:24], src[:, :, 24:48])
                    tB = lnt.tile([90, H, 12], F32, tag="rB", name="rB")
                    eng.tensor_add(tB[:], tA[:, :, 0:12], tA[:, :, 12:24])
                    tC = lnt.tile([90, H, 6], F32, tag="rC", name="rC")
                    eng.tensor_add(tC[:], tB[:, :, 0:6], tB[:, :, 6:12])
                    tD = lnt.tile([90, H, 3], F32, tag="rD", name="rD")
                    eng.tensor_add(tD[:], tC[:, :, 0:3], tC[:, :, 3:6])
                    eng.tensor_add(dst, tD[:, :, 0], tD[:, :, 1])
                    eng.tensor_add(dst, dst, tD[:, :, 2])

                for b in range(B):
                    for wc in range(2):
                        for ch in range(2):
                            zh = zhs[(b, wc, ch)]
                            sqh = spQ.tile([90, H, 48], F32, tag="QU", name="sqh")
                            nc.scalar.activation(out=sqh[:], in_=zh[:], func=AF.Square)
                            teng = (nc.gpsimd if (wc + ch) % 2 == 0
                                    else nc.vector) if b == 0 else nc.vector
                            if ch == 0:
                                nc.vector.reduce_sum(s_sum[b][:, wc, :], zh[:], axis=AX.X)
                                gp_reduce48(s_sq[b][:, wc, :], sqh, teng)
                            else:
                                nc.vector.reduce_sum(s_t[:], zh[:], axis=AX.X)
                                nc.vector.tensor_add(s_sum[b][:, wc, :], s_sum[b][:, wc, :], s_t[:])
                                sq2 = stp.tile([90, H], F32, tag="st_tmp2", name="s_tmp2")
                                gp_reduce48(sq2[:], sqh, teng)
                                teng.tensor_add(s_sq[b][:, wc, :], s_sq[b][:, wc, :], sq2[:])
                    nc.sync.dma_start(_stk(st_in[b], 0), s_sum[b][:])
                    nc.sync.dma_start(_stk(st_in[b], 1), s_sq[b][:])
                    nc.gpsimd.collective_compute(
                        "AllReduce", OP.add, replica_groups=rg,
                        ins=[st_in[b][:].opt()], outs=[st_out[b][:].opt()])

                for b in range(B):
                    nc.sync.dma_start(s_sum[b][:], _stk(st_out[b], 0))
                    nc.sync.dma_start(s_sq[b][:], _stk(st_out[b], 1))
                    nc.vector.tensor_scalar(out=s_m[b][:], in0=s_sum[b][:],
                                            scalar1=1.0 / C, scalar2=None,
                                            op0=OP.mult)
                    nc.vector.tensor_scalar(out=s_r[b][:], in0=s_sq[b][:],
                                            scalar1=1.0 / C, scalar2=None,
                                            op0=OP.mult)
                    tmp = stp.tile([90, 2, H], F32, tag=f"tmp{b}", name=f"tmpb{b}")
                    nc.vector.tensor_mul(tmp[:], s_m[b][:], s_m[b][:])
                    nc.vector.tensor_sub(s_r[b][:], s_r[b][:], tmp[:])
                    nc.scalar.activation(out=s_r[b][:], in_=s_r[b][:],
                                         func=AF.Sqrt, bias=c_eps[:90])
                    nc.vector.reciprocal(s_r[b][:], s_r[b][:])
                    # m*r rows for the rank-1 DFT correction of the LN mean
                    nc.vector.tensor_mul(s_mr[b][:], s_m[b][:], s_r[b][:])

                def _bcast48(t, b, wc):
                    # [90, H] stat slice broadcast along a trailing 48-axis
                    base = t[b][:]
                    return bass.AP(tensor=base.tensor,
                                   offset=base.offset + wc * H,
                                   ap=[[base.ap[0][0], 90], [1, H], [0, 48]])

                for b in range(B):
                    # ---- LN1 scale-only (z *= r, in place); the mean term
                    # is a channel-independent rank-1 correction applied at
                    # the F1 drain: yb = F1(z*r) - DFT_w(m*r).
                    yb = spA.tile([90, 92, BS], BF16, tag="YO", name="yb")
                    mrd = lnt.tile([90, 92], F32, tag="mrd", name="mrd")
                    pmr = pp.tile([90, 92], F32, tag="pp", name="psmr")
                    nc.tensor.matmul(pmr[:], s_mr[b][:, 0, :], c_fwp0[:],
                                     start=True, stop=False)
                    nc.tensor.matmul(pmr[:], s_mr[b][:, 1, :], c_fwp1[:],
                                     start=False, stop=True)
                    nc.vector.tensor_copy(mrd[:], pmr[:])
                    for ch in range(2):
                        mul_eng = nc.vector if ch == 0 else nc.gpsimd
                        for wc in range(2):
                            zt = zhs[(b, wc, ch)]
                            mul_eng.tensor_mul(zt[:], zt[:],
                                               _bcast48(s_r, b, wc))
                        zh0 = zhs[(b, 0, ch)]
                        zh1 = zhs[(b, 1, ch)]
                        # 5 output columns packed per PSUM bank -> 5x fewer
                        # (and 5x bigger) drains, fused with the -m*r fixup
                        for gi, g0 in enumerate(range(0, 48, 5)):
                            ncol = min(5, 48 - g0)
                            ps = pp.tile([90, 460], F32, tag="pp", name="psf1")
                            for idx in range(ncol):
                                cl_ = g0 + idx
                                sl = ps[:, idx * 92:(idx + 1) * 92]
                                nc.tensor.matmul(sl, zh0[:, :, cl_], c_fwp0[:],
                                                 start=True, stop=False)
                                nc.tensor.matmul(sl, zh1[:, :, cl_], c_fwp1[:],
                                                 start=False, stop=True)
                            c0 = ch * 48 + g0
                            src = bass.AP(tensor=ps[:].tensor,
                                          offset=ps[:].offset,
                                          ap=[[ps[:].ap[0][0], 90],
                                              [1, 92], [92, ncol]])
                            mrdv = bass.AP(tensor=mrd[:].tensor,
                                           offset=mrd[:].offset,
                                           ap=[[mrd[:].ap[0][0], 90],
                                               [1, 92], [0, ncol]])
                            nc.vector.tensor_sub(yb[:, :, c0:c0 + ncol],
                                                 src, mrdv)

                    # ---- F2 (5 wf packed per PSUM bank)
                    zb = spB.tile([BS, 2, KW, H], BF16, tag="ZO", name="zbt")
                    for g0 in range(0, KW, 5):
                        nwf = min(5, KW - g0)
                        prg = pp.tile([BS, 450], F32, tag="pp", name="psf2r")
                        pig = pp.tile([BS, 450], F32, tag="pp", name="psf2i")
                        for idx in range(nwf):
                            wf = g0 + idx
                            yr = yb[:, wf, :]
                            yi = yb[:, 46 + wf, :]
                            slr = prg[:, idx * 90:(idx + 1) * 90]
                            nc.tensor.matmul(slr, yr, c_fhc[:], start=True, stop=False)
                            nc.tensor.matmul(slr, yi, c_fhs[:], start=False, stop=True)
                            sli = pig[:, idx * 90:(idx + 1) * 90]
                            nc.tensor.matmul(sli, yi, c_fhc[:], start=True, stop=False)
                            nc.tensor.matmul(sli, yr, c_fhsm[:], start=False, stop=True)
                        nc.scalar.activation(out=zb[:, 0, g0:g0 + nwf, :],
                                             in_=prg[:, :nwf * 90],
                                             func=AF.Copy, scale=c_g1col[:])
                        nc.scalar.activation(out=zb[:, 1, g0:g0 + nwf, :],
                                             in_=pig[:, :nwf * 90],
                                             func=AF.Copy, scale=c_g1col[:])
                    nc.vector.tensor_scalar(out=zb[:, 0, 0, 0:1],
                                            in0=zb[:, 0, 0, 0:1],
                                            scalar1=c_spike[:], scalar2=None,
                                            op0=OP.add)

                    # ---- block MLP layer 1
                    o1 = spA.tile([BS + 1, 2, NM], BF16, tag="YO", name="o1t")
                    zr_f = zb[:, 0].rearrange("p a b -> p (a b)")
                    zi_f = zb[:, 1].rearrange("p a b -> p (a b)")
                    n0 = 0
                    while n0 < NM:
                        nn_ = min(512, NM - n0)
                        zr_s = zr_f[:, n0:n0 + nn_]
                        zi_s = zi_f[:, n0:n0 + nn_]
                        por = pp.tile([BS, 512], F32, tag="pp", name="pso1r")
                        nc.tensor.matmul(por[:, :nn_], c_w1r[:], zr_s,
                                         start=True, stop=False)
                        nc.tensor.matmul(por[:, :nn_], c_w1im[:], zi_s,
                                         start=False, stop=True)
                        poi = pp.tile([BS, 512], F32, tag="pp", name="pso1i")
                        nc.tensor.matmul(poi[:, :nn_], c_w1i[:], zr_s,
                                         start=True, stop=False)
                        nc.tensor.matmul(poi[:, :nn_], c_w1r[:], zi_s,
                                         start=False, stop=True)
                        nc.scalar.activation(out=o1[0:BS, 0, n0:n0 + nn_],
                                             in_=por[:, :nn_], func=AF.Relu,
                                             bias=c_b1r[:])
                        nc.scalar.activation(out=o1[0:BS, 1, n0:n0 + nn_],
                                             in_=poi[:, :nn_], func=AF.Relu,
                                             bias=c_b1i[:])
                        n0 += nn_

                    # ---- block MLP layer 2 + softshrink (5 wf packed/bank)
                    o2 = spB.tile([H, 2, KW, BS], BF16, tag="ZO", name="o2t")
                    o1r_f = o1[:, 0]
                    o1i_f = o1[:, 1]
                    for g0 in range(0, KW, 5):
                        nwf = min(5, KW - g0)
                        prg = pp.tile([H, 480], F32, tag="pp", name="pso2r")
                        pig = pp.tile([H, 480], F32, tag="pp", name="pso2i")
                        for idx in range(nwf):
                            wf = g0 + idx
                            lr = o1r_f[0:BS, wf * H:(wf + 1) * H]
                            li = o1i_f[0:BS, wf * H:(wf + 1) * H]
                            slr = prg[:, idx * BS:(idx + 1) * BS]
                            nc.tensor.matmul(slr, lr, c_w2a[0:BS, :], start=True, stop=False)
                            nc.tensor.matmul(slr, li, c_w2b[0:BS, :], start=False, stop=False)
                            nc.tensor.matmul(slr, c_ones1b[:, 0:H], c_b2rr[:], start=False, stop=True)
                            sli = pig[:, idx * BS:(idx + 1) * BS]
                            nc.tensor.matmul(sli, li, c_w2c[0:BS, :], start=True, stop=False)
                            nc.tensor.matmul(sli, lr, c_w2d[0:BS, :], start=False, stop=False)
                            nc.tensor.matmul(sli, c_ones1b[:, 0:H], c_b2ir[:], start=False, stop=True)
                        for ri, psm in ((0, prg), (1, pig)):
                            clip = clp.tile([H, 480], F32, tag="clip", name="clipt")
                            nc.vector.tensor_scalar(out=clip[:, :nwf * BS],
                                                    in0=psm[:, :nwf * BS],
                                                    scalar1=-LAM, scalar2=LAM,
                                                    op0=OP.max, op1=OP.min)
                            nc.vector.tensor_sub(o2[:, ri, g0:g0 + nwf, :],
                                                 psm[:, :nwf * BS],
                                                 clip[:, :nwf * BS])

                    # ---- inverse H-DFT -> u2r/u2i [46, (c, h)] (5 c / bank)
                    u2r = spQ.tile([KW, BS, H], BF16, tag="QU", name="u2rt")
                    u2i = spA.tile([KW, BS, H], BF16, tag="YO", name="u2it")
                    for g0 in range(0, BS, 5):
                        ncl = min(5, BS - g0)
                        purg = pp.tile([KW, 450], F32, tag="pp", name="psur")
                        puig = pp.tile([KW, 450], F32, tag="pp", name="psui")
                        for idx in range(ncl):
                            c = g0 + idx
                            lr = o2[:, 0, :, c]
                            li = o2[:, 1, :, c]
                            slr = purg[:, idx * H:(idx + 1) * H]
                            nc.tensor.matmul(slr, lr, c_fhc[:], start=True, stop=False)
                            nc.tensor.matmul(slr, li, c_fhsm[:], start=False, stop=True)
                            sli = puig[:, idx * H:(idx + 1) * H]
                            nc.tensor.matmul(sli, li, c_fhc[:], start=True, stop=False)
                            nc.tensor.matmul(sli, lr, c_fhs[:], start=False, stop=True)
                        nc.scalar.activation(out=u2r[:, g0:g0 + ncl, :],
                                             in_=purg[:, :ncl * H], func=AF.Copy)
                        nc.vector.tensor_copy(u2i[:, g0:g0 + ncl, :],
                                              puig[:, :ncl * H])

                    # ---- inverse W-DFT -> SBUF gather sbA (bf16), 2 c/bank
                    sbA = sbg.tile([H, BS, W], BF16, tag="sbA", name="sbA")
                    for c0 in range(0, BS, 2):
                        pf = pp.tile([H, 360], F32, tag="pp", name="psf")
                        for idx in range(2):
                            c = c0 + idx
                            sl = pf[:, idx * W:(idx + 1) * W]
                            nc.tensor.matmul(sl, u2r[:, c, :], c_iwrt[:],
                                             start=True, stop=False)
                            nc.tensor.matmul(sl, u2i[:, c, :], c_iwit[:],
                                             start=False, stop=True)
                        if (c0 // 2) % 2 == 0:
                            nc.scalar.activation(out=sbA[:, c0:c0 + 2, :],
                                                 in_=pf[:], func=AF.Copy)
                        else:
                            nc.vector.tensor_copy(sbA[:, c0:c0 + 2, :], pf[:])

                    # ---- a2a send pieces (SBUF -> DRAM), quarter-major
                    # so quarter 0's collective can fire first
                    for q in range(4):
                        qw = qws[q]
                        for jq in range(4):
                            j = b * 4 + jq
                            for (ha, hb_, wa, wb_, toff) in _shard_pieces(
                                    jq, QB[q], QB[q + 1]):
                                src = sbA[ha:hb_, :, wa:wb_]
                                dst = bass.AP(
                                    tensor=a2a_in[q][:].tensor,
                                    offset=a2a_in[q][:].offset
                                    + (j * BS * qw + toff),
                                    ap=[[wb_ - wa, hb_ - ha], [qw, BS],
                                        [1, wb_ - wa]])
                                nc.sync.dma_start(dst, src)

            for q in range(4):
                nc.gpsimd.collective_compute(
                    "AllToAll", OP.bypass, replica_groups=rg,
                    ins=[a2a_in[q][:].opt()], outs=[a2a_out[q][:].opt()])

            # ================= phase 2 =================
            with ExitStack() as _st2:
                fc1p = _st2.enter_context(tc.tile_pool(name="fc1p", bufs=1))
                fc2p = _st2.enter_context(tc.tile_pool(name="fc2p", bufs=1))
                lnp = _st2.enter_context(tc.tile_pool(name="lnp", bufs=1))

                FP8 = mybir.dt.float8e4
                c_fc1 = [fc1p.tile([128, 2, HID], FP8, tag=f"fc1_{i}",
                                   name=f"cfc1_{i}") for i in range(3)]
                for i in range(3):
                    nc.sync.dma_start(c_fc1[i][:], fc1q[i])
                c_fc2 = [fc2p.tile([128, 2, C], FP8, tag=f"fc2_{i}",
                                   name=f"cfc2_{i}") for i in range(12)]
                for i in range(12):
                    nc.sync.dma_start(c_fc2[i][:], fc2q[i])

                R1B = lnp.tile([128, TSH], BF16, tag="R1B", name="R1B")
                MR1B = lnp.tile([128, TSH], BF16, tag="MR1B", name="MR1B")

                with ExitStack() as _stR:
                    rowp = _stR.enter_context(tc.tile_pool(name="rowp", bufs=1))
                    pbR = _stR.enter_context(
                        tc.tile_pool(name="pbR", bufs=2, space="PSUM"))
                    # all-shard LN1 stats rows [8, 4050], computed in place
                    r1_8 = rowp.tile([NCORES, TSH], F32, tag="r18", name="r18")
                    mr1_8 = rowp.tile([NCORES, TSH], F32, tag="mr18", name="mr18")
                    rtmp = rowp.tile([NCORES, TSH], F32, tag="rtmp", name="rtmp")
                    for kind, dstt in ((0, rtmp), (1, r1_8)):
                        for s in range(NCORES):
                            bb, jq = s // 4, s % 4
                            for (ha, hb_, wa, wb_, toff) in _shard_pieces(jq):
                                src_ = bass.AP(
                                    tensor=st_out[bb][:].tensor,
                                    offset=st_out[bb][:].offset
                                    + (kind * W * H + wa * H + ha),
                                    ap=[[0, 1], [1, hb_ - ha], [H, wb_ - wa]])
                                nc.sync.dma_start(
                                    dstt[s:s + 1,
                                         toff:toff + (hb_ - ha) * (wb_ - wa)],
                                    src_)
                    nc.vector.tensor_scalar(out=mr1_8[:], in0=rtmp[:],
                                            scalar1=1.0 / C, scalar2=None,
                                            op0=OP.mult)           # m1
                    nc.vector.tensor_scalar(out=r1_8[:], in0=r1_8[:],
                                            scalar1=1.0 / C, scalar2=None,
                                            op0=OP.mult)           # q/C
                    nc.vector.tensor_mul(rtmp[:], mr1_8[:], mr1_8[:])
                    nc.vector.tensor_sub(r1_8[:], r1_8[:], rtmp[:])  # var
                    nc.scalar.activation(out=r1_8[:], in_=r1_8[:], func=AF.Sqrt,
                                         bias=c_eps[:NCORES])
                    nc.vector.reciprocal(r1_8[:], r1_8[:])           # r1
                    nc.vector.tensor_mul(mr1_8[:], mr1_8[:], r1_8[:])  # m1*r1

                    # pre-broadcast this core's r1 / m1*r1 to bf16 planes
                    for it, T in enumerate(TTS):
                        t0 = TT0[it]
                        for rows, plane in ((r1_8, R1B), (mr1_8, MR1B)):
                            pb = pbR.tile([128, 512], F32, tag="pbc")
                            nc.tensor.matmul(pb[:, :T], c_mask[:],
                                             rows[:, t0:t0 + T],
                                             start=True, stop=True)
                            nc.scalar.activation(out=plane[:, t0:t0 + T],
                                                 in_=pb[:, :T], func=AF.Copy)

                xcp = _st2.enter_context(tc.tile_pool(name="xcp", bufs=6))
                rcvp = _st2.enter_context(tc.tile_pool(name="rcvp", bufs=6))
                t1p = _st2.enter_context(tc.tile_pool(name="t1p", bufs=3))
                htokp = _st2.enter_context(tc.tile_pool(name="htokp", bufs=12))
                htnp = _st2.enter_context(tc.tile_pool(name="htnp", bufs=12))
                hidp = _st2.enter_context(tc.tile_pool(name="hidp", bufs=1))
                rw2 = _st2.enter_context(tc.tile_pool(name="rw2", bufs=2))
                bcp = _st2.enter_context(tc.tile_pool(name="bcp", bufs=2))
                outp = _st2.enter_context(tc.tile_pool(name="outp", bufs=3))
                ph = _st2.enter_context(tc.tile_pool(name="ph", bufs=2, space="PSUM"))
                po = _st2.enter_context(tc.tile_pool(name="po", bufs=2, space="PSUM"))
                pst = _st2.enter_context(tc.tile_pool(name="pst", bufs=2, space="PSUM"))
                pbc = _st2.enter_context(tc.tile_pool(name="pbc", bufs=2, space="PSUM"))

                # software-pipelined chunk loop: front half (loads, assemble,
                # LN2 stats, normalize) runs one chunk ahead of the back half
                # (fc1 -> gelu -> fc2 -> residual -> store).
                hts = {}
                htns = {}
                hid = hidp.tile([128, NMO, 512], FP8, tag="hid", name="hid")
                for i in range(NCH + 1):
                    if i < NCH:
                        T = TTS[i]
                        t0 = TT0[i]
                        htoks = []
                        htnl = []
                        for cc in range(NCC):
                            xct = xcp.tile([128, 512], F32, tag="xct")
                            nc.sync.dma_start(xct[:, :T],
                                              xc[cc * 128:(cc + 1) * 128, t0:t0 + T])
                            rcv = rcvp.tile([128, 512], BF16, tag="rcv")
                            q = QMAP[i]
                            lt0 = t0 - QB[q]
                            c0 = cc * 128
                            r0 = 0
                            while r0 < 128:
                                s_blk = (c0 + r0) // BS
                                c_in = (c0 + r0) % BS
                                nrow = min(BS - c_in, 128 - r0)
                                nc.sync.dma_start(
                                    rcv[r0:r0 + nrow, :T],
                                    a2a_out[q][s_blk, c_in:c_in + nrow,
                                               lt0:lt0 + T])
                                r0 += nrow
                            ht = htokp.tile([128, 512], F32, tag="htok")
                            htoks.append(ht)
                            t1 = t1p.tile([128, 512], F32, tag="t1")
                            nc.gpsimd.tensor_mul(t1[:, :T], xct[:, :T],
                                                 R1B[:, t0:t0 + T])
                            nc.vector.tensor_sub(t1[:, :T], t1[:, :T],
                                                 MR1B[:, t0:t0 + T])
                            nc.vector.tensor_scalar(out=t1[:, :T], in0=t1[:, :T],
                                                    scalar1=c_g1f[:, cc:cc + 1],
                                                    scalar2=c_be1f[:, cc:cc + 1],
                                                    op0=OP.mult, op1=OP.add)
                            nc.gpsimd.tensor_add(ht[:, :T], rcv[:, :T], xct[:, :T])
                            nc.vector.tensor_add(ht[:, :T], ht[:, :T], t1[:, :T])

                        # LN2 stats via bf16 ones-matmul (1 cyc/row vs 4)
                        ps_s = pst.tile([1, 512], F32, tag="pst")
                        ps_q = pst.tile([1, 512], F32, tag="pst")
                        htbs = []
                        for cc in range(NCC):
                            htb = t1p.tile([128, 512], BF16, tag="htb", name="htb")
                            nc.scalar.activation(out=htb[:, :T],
                                                 in_=htoks[cc][:, :T],
                                                 func=AF.Copy)
                            htbs.append(htb)
                        for cc in range(NCC):
                            nc.tensor.matmul(ps_s[:, :T], c_ones128b[:],
                                             htbs[cc][:, :T],
                                             start=(cc == 0), stop=(cc == NCC - 1))
                        hsqs = []
                        for cc in range(NCC):
                            hsq = t1p.tile([128, 512], BF16, tag="hsq", name="hsq")
                            heng = nc.gpsimd if cc % 2 == 0 else nc.vector
                            heng.tensor_mul(hsq[:, :T], htbs[cc][:, :T],
                                            htbs[cc][:, :T])
                            hsqs.append(hsq)
                        for cc in range(NCC):
                            nc.tensor.matmul(ps_q[:, :T], c_ones128b[:],
                                             hsqs[cc][:, :T],
                                             start=(cc == 0), stop=(cc == NCC - 1))
                        m2r = rw2.tile([1, 512], F32, tag="m2r")
                        r2r = rw2.tile([1, 512], F32, tag="r2r")
                        vv = rw2.tile([1, 512], F32, tag="vv")
                        nc.vector.tensor_scalar(out=m2r[:, :T], in0=ps_s[:, :T],
                                                scalar1=1.0 / C, scalar2=None,
                                                op0=OP.mult)
                        nc.vector.tensor_scalar(out=r2r[:, :T], in0=ps_q[:, :T],
                                                scalar1=1.0 / C, scalar2=None,
                                                op0=OP.mult)
                        nc.vector.tensor_mul(vv[:, :T], m2r[:, :T], m2r[:, :T])
                        nc.vector.tensor_sub(r2r[:, :T], r2r[:, :T], vv[:, :T])
                        nc.scalar.activation(out=r2r[:, :T], in_=r2r[:, :T],
                                             func=AF.Sqrt, bias=c_eps[:1])
                        nc.vector.reciprocal(r2r[:, :T], r2r[:, :T])
                        # broadcast m2, r2 to all partitions
                        m2b = bcp.tile([128, 512], F32, tag="m2b")
                        r2b = bcp.tile([128, 512], F32, tag="r2b")
                        for rowt, bt in ((m2r, m2b), (r2r, r2b)):
                            pb = pbc.tile([128, 512], F32, tag="pbc")
                            nc.tensor.matmul(pb[:, :T], c_ones1[:], rowt[:, :T],
                                             start=True, stop=True)
                            nc.scalar.activation(out=bt[:, :T], in_=pb[:, :T],
                                                 func=AF.Copy)
                        # normalize -> fp8e4m3 (DoubleRow slot layout)
                        for cc in range(NCC):
                            if cc % 2 == 0:
                                htn = htnp.tile([128, 2, 512], FP8, tag="htn")
                                htnl.append(htn)
                            tn = t1p.tile([128, 512], F32, tag="t1", name="tn")
                            nc.vector.tensor_sub(tn[:, :T], htoks[cc][:, :T],
                                                 m2b[:, :T])
                            nc.vector.tensor_mul(htn[:, cc % 2, :T], tn[:, :T],
                                                 r2b[:, :T])
                        hts[i] = htoks
                        htns[i] = htnl

                    if i >= 1:
                        j = i - 1
                        T = TTS[j]
                        t0 = TT0[j]
                        htoks = hts.pop(j)
                        htnl = htns.pop(j)
                        # fc1 (fp8 DoubleRow) + gelu -> hid (fp8)
                        for mo in range(NMO):
                            php = ph.tile([128, 512], F32, tag="ph")
                            for p_ in range(3):
                                nc.tensor.matmul(
                                    php[:, :T],
                                    c_fc1[p_][:, :, mo * 128:(mo + 1) * 128],
                                    htnl[p_][:, :, :T],
                                    start=(p_ == 0), stop=(p_ == 2),
                                    perf_mode=mybir.MatmulPerfMode.DoubleRow)
                            nc.scalar.activation(out=hid[:, mo, :T],
                                                 in_=php[:, :T], func=AF.Gelu,
                                                 scale=1.0 / WSCALE,
                                                 bias=c_gbias[:, mo:mo + 1])
                        # fc2 (fp8 DoubleRow) + bias + residual -> store
                        for co in range(NCC):
                            pop = po.tile([128, 512], F32, tag="po")
                            for p_ in range(12):
                                nc.tensor.matmul(
                                    pop[:, :T],
                                    c_fc2[p_][:, :, co * 128:(co + 1) * 128],
                                    hid[:, 2 * p_:2 * p_ + 2, :T],
                                    start=(p_ == 0), stop=(p_ == 11),
                                    perf_mode=mybir.MatmulPerfMode.DoubleRow)
                            osb = outp.tile([128, 512], F32, tag="osb")
                            nc.scalar.activation(out=osb[:, :T],
                                                 in_=pop[:, :T],
                                                 func=AF.Identity,
                                                 scale=1.0 / WSCALE,
                                                 bias=c_fc2b[:, co:co + 1])
                            nc.vector.tensor_add(osb[:, :T], osb[:, :T],
                                                 htoks[co][:, :T])
                            nc.sync.dma_start(
                                out[co * 128:(co + 1) * 128, t0:t0 + T],
                                osb[:, :T])

    nc.compile()
    _CACHE["nc"] = nc
    return nc


def _host_prep(inputs):
    x = np.ascontiguousarray(np.asarray(inputs["x"], dtype=np.float32))
    g1 = np.asarray(inputs["g1"], np.float32); be1 = np.asarray(inputs["be1"], np.float32)
    g2 = np.asarray(inputs["g2"], np.float32); be2 = np.asarray(inputs["be2"], np.float32)
    w1 = np.asarray(inputs["w1"], np.float32); b1 = np.asarray(inputs["b1"], np.float32)
    w2 = np.asarray(inputs["w2"], np.float32); b2 = np.asarray(inputs["b2"], np.float32)
    fc1_w = np.asarray(inputs["fc1_w"], np.float32)
    fc1_b = np.asarray(inputs["fc1_b"], np.float32)
    fc2_w = np.asarray(inputs["fc2_w"], np.float32)
    fc2_b = np.asarray(inputs["fc2_b"], np.float32)

    fwp0, fwp1, fhc_m, fhs_m, fhsm_m, iwrt_m, iwit_m = _dft_consts()
    xf = x.reshape(TOK, C)
    bf = ml_dtypes.bfloat16
    f8 = ml_dtypes.float8_e4m3fn
    fc1q_m = np.ascontiguousarray(
        (g2[:, None] * fc1_w * WSCALE).reshape(3, 2, 128, HID)
        .transpose(0, 2, 1, 3)).astype(f8)                     # (3,128,2,3072)
    fc2q_m = np.ascontiguousarray(
        (fc2_w * WSCALE).reshape(12, 2, 128, C)
        .transpose(0, 2, 1, 3)).astype(f8)                     # (12,128,2,768)
    gbias_v = (fc1_b + be2 @ fc1_w).astype(np.float32)         # (3072,)
    gbias_m = np.ascontiguousarray(gbias_v.reshape(NMO, 128).T)  # (128, 24)
    fc2b_m = np.ascontiguousarray(fc2_b.reshape(NCC, 128).T)
    g1f_m = np.ascontiguousarray(g1.reshape(NCC, 128).T)
    be1f_m = np.ascontiguousarray(be1.reshape(NCC, 128).T)
    ones1 = np.ones((1, 128), np.float32)
    ones1b = np.ones((1, 128), bf)
    ones128 = np.ones((128, 1), np.float32)
    ones128b = np.ones((128, 1), bf)

    in_maps = []
    for k in range(NCORES):
        ck = slice(k * BS, (k + 1) * BS)
        # (W, B, H, 96) -> (W, B, 2, H, 48), bf16
        xw_k = np.ascontiguousarray(
            x[:, :, :, ck].transpose(2, 0, 1, 3)
            .reshape(W, B, H, 2, 48).transpose(0, 1, 3, 2, 4)).astype(bf)
        xc_k = np.ascontiguousarray(xf[k * TSH:(k + 1) * TSH, :].T)
        w1r_k = np.ascontiguousarray(w1[k, :, :, 0])
        w1i_k = np.ascontiguousarray(w1[k, :, :, 1])
        w2r_k = np.ascontiguousarray(w2[k, :, :, 0])
        w2i_k = np.ascontiguousarray(w2[k, :, :, 1])
        b2r_k = b2[k, :, 0]; b2i_k = b2[k, :, 1]
        zr = np.zeros((1, BS), np.float32)
        mask = np.zeros((NCORES, 128), np.float32); mask[k, :] = 1.0
        in_maps.append({
            "xw": xw_k, "xc": xc_k,
            "fwp0": fwp0.astype(bf), "fwp1": fwp1.astype(bf),
            "fhc": fhc_m.astype(bf), "fhs": fhs_m.astype(bf),
            "fhsm": fhsm_m.astype(bf),
            "iwrt": iwrt_m.astype(bf), "iwit": iwit_m.astype(bf),
            "w1r": w1r_k.astype(bf), "w1i": w1i_k.astype(bf),
            "w1im": (-w1i_k).astype(bf),
            "b1r": b1[k, :, 0:1].copy(), "b1i": b1[k, :, 1:2].copy(),
            "w2a": np.concatenate([w2r_k, b2r_k[None, :]], 0).astype(bf),
            "w2b": np.concatenate([-w2i_k, zr], 0).astype(bf),
            "w2c": np.concatenate([w2r_k, zr], 0).astype(bf),
            "w2d": np.concatenate([w2i_k, b2i_k[None, :]], 0).astype(bf),
            "g1col": g1[ck][:, None].copy(),
            "b2rr": b2r_k[None, :].astype(bf), "b2ir": b2i_k[None, :].astype(bf),
            "spike": (be1[ck] * SQN)[:, None].astype(np.float32),
            "fc1q": fc1q_m, "fc2q": fc2q_m, "gbias": gbias_m,
            "fc2b": fc2b_m, "g1f": g1f_m, "be1f": be1f_m,
            "ones1": ones1, "ones1b": ones1b, "ones128": ones128,
            "ones128b": ones128b,
            "mask128": mask,
        })
    return in_maps


def kernel(**inputs):
    nc = _build_nc()
    in_maps = _host_prep(inputs)
    res = run_bass_kernel_spmd(nc, in_maps, core_ids=list(range(NCORES)))
    outs = [np.asarray(res.results[j]["out"], dtype=np.float32).T
            for j in range(NCORES)]
    full = np.concatenate(outs, axis=0).reshape(B, H, W, C)
    return np.ascontiguousarray(full, dtype=np.float32)


# revision 41
# speedup vs baseline: 1.0578x; 1.0009x over previous
"""AFNO transformer block on 8 Trainium2 NeuronCores.

Distribution:
  Phase 1 (channel-block sharded): core k owns channels [96k, 96k+96).
    LN1 stats partial sums -> per-batch AllReduce -> LN1 apply, then the
    whole spectral path (rFFT2 as DFT matmuls, block-diagonal complex MLP,
    inverse rFFT2) entirely core-local. DFTs use "flip" matmuls (data as
    the stationary operand) so every stage lands in the layout the next
    stage contracts over - no on-chip transposes.
  AllToAll (bf16 payload): filter output reshard (channel -> token).
  Phase 2 (token sharded): core j owns tokens [4050j, 4050j+4050).
    h = filt + LN1(x) + x assembled channel-major; LN1 per-token scales
    pre-broadcast to bf16 [128, 4050] planes via mask matmuls. LN2 stats
    by ones-matmul, normalize-first (htn = (h-m2)*r2 in bf16), then
    bf16 fc1 (g2-folded) -> exact Gelu -> bf16 fc2 -> residual. Weights
    for fc1 AND fc2 stay resident in SBUF in bf16. Output is written
    channel-major [C, TSH]; the host transposes.
"""
import math
import numpy as np
import ml_dtypes

import concourse.bass as bass
import concourse.mybir as mybir
import concourse.tile as tile
from concourse import bacc
from concourse.bass_utils import run_bass_kernel_spmd

F32 = mybir.dt.float32
F32R = mybir.dt.float32r
BF16 = mybir.dt.bfloat16
AF = mybir.ActivationFunctionType
OP = mybir.AluOpType
AX = mybir.AxisListType

NCORES = 8
B, H, W, C = 2, 90, 180, 768
BS = 96           # channels per core / AFNO block size
KW = 46           # kept W-frequency modes
HID = 3072
LAM = 0.01
EPS = 1e-5
TOK = B * H * W   # 32400
TSH = TOK // NCORES  # 4050
NM = KW * H       # modes per batch elem: 4140
SQN = math.sqrt(H * W)

QB = [0, 506, 1519, 2532, 4050]      # a2a quarter boundaries (tokens)
TTS = [506, 507, 506, 507, 506, 506, 506, 506]
QMAP = [0, 1, 1, 2, 2, 3, 3, 3]      # chunk -> a2a quarter
TT0 = [sum(TTS[:i]) for i in range(len(TTS))]
NCH = len(TTS)
NCC = 6    # 768/128
WSCALE = 64.0  # fp8 weight pre-scale for fc1/fc2
NMO = 24   # 3072/128


def _dft_consts():
    wv = np.arange(W, dtype=np.float64)[:, None]
    wf = np.arange(KW, dtype=np.float64)[None, :]
    ang = 2.0 * np.pi * wv * wf / W
    fwr = np.cos(ang) / math.sqrt(W)
    fwi = -np.sin(ang) / math.sqrt(W)
    fwpack = np.concatenate([fwr, fwi], axis=1)          # (180, 92)
    hv = np.arange(H, dtype=np.float64)[:, None]
    hf = np.arange(H, dtype=np.float64)[None, :]
    angh = 2.0 * np.pi * hv * hf / H
    fhc = np.cos(angh) / math.sqrt(H)                    # symmetric
    fhs = np.sin(angh) / math.sqrt(H)
    alpha = np.ones(KW); alpha[1:] = 2.0
    iwr = alpha[None, :] * np.cos(ang) / math.sqrt(W)    # (180, 46)
    iwi = -alpha[None, :] * np.sin(ang) / math.sqrt(W)
    f32 = np.float32
    return (fwpack[:90].astype(f32), fwpack[90:].astype(f32),
            fhc.astype(f32), fhs.astype(f32), (-fhs).astype(f32),
            np.ascontiguousarray(iwr.T).astype(f32),
            np.ascontiguousarray(iwi.T).astype(f32))


def _shard_pieces(jq, lo=0, hi=TSH):
    """(ha,hb,wa,wb,tok_off) pieces of within-batch shard jq, tokens
    [lo, hi) of the shard; tok_off is relative to lo."""
    s0, e0 = TSH * jq + lo, TSH * jq + hi
    pieces, t = [], s0
    while t < e0:
        h = t // W
        wa = t - h * W
        if wa != 0 or e0 - t < W:
            wb = min(W, wa + (e0 - t))
            pieces.append((h, h + 1, wa, wb, t - s0))
            t += wb - wa
        else:
            hb = min(H, h + (e0 - t) // W)
            pieces.append((h, hb, 0, W, t - s0))
            t += (hb - h) * W
    return pieces


_CACHE = {}


def _build_nc():
    if "nc" in _CACHE:
        return _CACHE["nc"]
    nc = bacc.Bacc("TRN2", target_bir_lowering=False, debug=False,
                   num_devices=NCORES)
    g = lambda n, s, dt=F32: nc.dram_tensor(n, s, dt, kind="ExternalInput")
    xw = g("xw", [W, B, 2, H, 48], BF16)
    xc = g("xc", [C, TSH])
    fwp0 = g("fwp0", [90, 92], BF16); fwp1 = g("fwp1", [90, 92], BF16)
    fhc = g("fhc", [90, 90], BF16); fhs = g("fhs", [90, 90], BF16)
    fhsm = g("fhsm", [90, 90], BF16)
    iwrt = g("iwrt", [KW, W], BF16); iwit = g("iwit", [KW, W], BF16)
    w1r = g("w1r", [BS, BS], BF16); w1i = g("w1i", [BS, BS], BF16)
    w1im = g("w1im", [BS, BS], BF16)
    b1r = g("b1r", [BS, 1]); b1i = g("b1i", [BS, 1])
    w2a = g("w2a", [BS + 1, BS], BF16); w2b = g("w2b", [BS + 1, BS], BF16)
    w2c = g("w2c", [BS + 1, BS], BF16); w2d = g("w2d", [BS + 1, BS], BF16)
    g1col = g("g1col", [BS, 1]); spike = g("spike", [BS, 1])
    b2rr = g("b2rr", [1, BS], BF16); b2ir = g("b2ir", [1, BS], BF16)
    ones1b = g("ones1b", [1, 128], BF16)
    FP8 = mybir.dt.float8e4
    fc1q = g("fc1q", [3, 128, 2, HID], FP8)
    fc2q = g("fc2q", [12, 128, 2, C], FP8)
    gbias = g("gbias", [128, NMO])
    fc2b = g("fc2b", [128, NCC])
    g1f = g("g1f", [128, NCC]); be1f = g("be1f", [128, NCC])
    ones1 = g("ones1", [1, 128])
    ones128 = g("ones128", [128, 1])
    ones128b = g("ones128b", [128, 1], BF16)
    mask128 = g("mask128", [NCORES, 128])   # one-hot row = this core's shard

    out = nc.dram_tensor("out", [C, TSH], F32, kind="ExternalOutput")
    rg = [list(range(NCORES))]

    from contextlib import ExitStack
    with tile.TileContext(nc) as tc:
        with ExitStack() as _st0:
            cp = _st0.enter_context(tc.tile_pool(name="const", bufs=1))
            dram = _st0.enter_context(tc.tile_pool(name="dram", bufs=1, space="DRAM"))
            def cl(t, shape, dt=F32):
                nm = f"c_{t.name if hasattr(t, 'name') else t[:].tensor.name}"
                s = cp.tile(shape, dt, name=nm, tag=nm)
                nc.scalar.dma_start(s[:], t[:])
                return s
            c_fwp0 = cl(fwp0, [90, 92], BF16); c_fwp1 = cl(fwp1, [90, 92], BF16)
            c_fhc = cl(fhc, [90, 90], BF16); c_fhs = cl(fhs, [90, 90], BF16)
            c_fhsm = cl(fhsm, [90, 90], BF16)
            c_iwrt = cl(iwrt, [KW, W], BF16); c_iwit = cl(iwit, [KW, W], BF16)
            c_w1r = cl(w1r, [BS, BS], BF16); c_w1i = cl(w1i, [BS, BS], BF16)
            c_w1im = cl(w1im, [BS, BS], BF16)
            c_b1r = cl(b1r, [BS, 1]); c_b1i = cl(b1i, [BS, 1])
            c_w2a = cl(w2a, [BS + 1, BS], BF16); c_w2b = cl(w2b, [BS + 1, BS], BF16)
            c_w2c = cl(w2c, [BS + 1, BS], BF16); c_w2d = cl(w2d, [BS + 1, BS], BF16)
            c_g1col = cl(g1col, [BS, 1]); c_spike = cl(spike, [BS, 1])
            c_b2rr = cl(b2rr, [1, BS], BF16); c_b2ir = cl(b2ir, [1, BS], BF16)
            c_ones1b = cl(ones1b, [1, 128], BF16)
            c_ones1 = cl(ones1, [1, 128]); c_ones128 = cl(ones128, [128, 1])
            c_ones128b = cl(ones128b, [128, 1], BF16)
            c_gbias = cl(gbias, [128, NMO]); c_fc2b = cl(fc2b, [128, NCC])
            c_g1f = cl(g1f, [128, NCC]); c_be1f = cl(be1f, [128, NCC])
            c_mask = cl(mask128, [NCORES, 128])
            c_eps = cp.tile([128, 1], F32, name="c_eps")
            nc.vector.memset(c_eps[:], EPS)

            qws = [QB[q + 1] - QB[q] for q in range(4)]
            a2a_in = [dram.tile([NCORES, BS, qws[q]], BF16, name=f"a2ai{q}")
                      for q in range(4)]
            a2a_out = [dram.tile([NCORES, BS, qws[q]], BF16, name=f"a2ao{q}")
                       for q in range(4)]
            st_in = [dram.tile([2, W, H], BF16, name=f"st_in{b_}") for b_ in range(B)]
            st_out = [dram.tile([2, W, H], BF16, name=f"st_out{b_}") for b_ in range(B)]

            # ================= phase 1 =================
            with ExitStack() as _st1:
                stp = _st1.enter_context(tc.tile_pool(name="stats", bufs=1))
                zp = _st1.enter_context(tc.tile_pool(name="zp", bufs=1))
                lnt = _st1.enter_context(tc.tile_pool(name="lnt", bufs=2))
                spA = _st1.enter_context(tc.tile_pool(name="spA", bufs=1))
                spB = _st1.enter_context(tc.tile_pool(name="spB", bufs=1))
                spQ = _st1.enter_context(tc.tile_pool(name="spQ", bufs=1))
                sbg = _st1.enter_context(tc.tile_pool(name="sbg", bufs=1))
                clp = _st1.enter_context(tc.tile_pool(name="clipp", bufs=2))
                pp = _st1.enter_context(tc.tile_pool(name="psum1", bufs=8, space="PSUM"))

                s_sum = [stp.tile([90, 2, H], F32, tag=f"ss{b_}", name=f"ssum{b_}") for b_ in range(B)]
                s_sq = [stp.tile([90, 2, H], F32, tag=f"sq{b_}", name=f"ssq{b_}") for b_ in range(B)]
                s_m = [stp.tile([90, 2, H], F32, tag=f"sm{b_}", name=f"sm{b_}") for b_ in range(B)]
                s_r = [stp.tile([90, 2, H], F32, tag=f"sr{b_}", name=f"sr{b_}") for b_ in range(B)]
                s_mr = [stp.tile([90, 2, H], BF16, tag=f"smr{b_}", name=f"smr{b_}") for b_ in range(B)]
                s_t = stp.tile([90, H], F32, tag="st_tmp", name="s_tmp")

                # resident bf16 x tiles: zhs[(b, wc, ch)] = [90, H, 48]
                zhs = {}
                for b in range(B):
                    for wc in range(2):
                        for ch in range(2):
                            zh = zp.tile([90, H, 48], BF16, tag=f"z{b}{wc}{ch}",
                                         name=f"zh{b}{wc}{ch}")
                            nc.sync.dma_start(zh[:], xw[wc * 90:(wc + 1) * 90, b, ch])
                            zhs[(b, wc, ch)] = zh

                # ---- stats pass (both b) + AllReduce per b
                def _stk(t, kind):
                    return bass.AP(tensor=t[:].tensor,
                                   offset=t[:].offset + kind * W * H,
                                   ap=[[90, 90], [8100, 2], [1, 90]])

                def gp_reduce48(dst, src, eng):
                    # sum src [90, H, 48] over last axis into dst [90, H],
                    # as a halving tree on the given engine
                    tA = lnt.tile([90, H, 24], F32, tag="rA", name="rA")
                    eng.tensor_add(tA[:], src[:, :, 0:24], src[:, :, 24:48])
                    tB = lnt.tile([90, H, 12], F32, tag="rB", name="rB")
                    eng.tensor_add(tB[:], tA[:, :, 0:12], tA[:, :, 12:24])
                    tC = lnt.tile([90, H, 6], F32, tag="rC", name="rC")
                    eng.tensor_add(tC[:], tB[:, :, 0:6], tB[:, :, 6:12])
                    tD = lnt.tile([90, H, 3], F32, tag="rD", name="rD")
                    eng.tensor_add(tD[:], tC[:, :, 0:3], tC[:, :, 3:6])
                    eng.tensor_add(dst, tD[:, :, 0], tD[:, :, 1])
                    eng.tensor_add(dst, dst, tD[:, :, 2])

                for b in range(B):
                    for wc in range(2):
                        for ch in range(2):
                            zh = zhs[(b, wc, ch)]
                            sqh = spQ.tile([90, H, 48], F32, tag="QU", name="sqh")
                            nc.scalar.activation(out=sqh[:], in_=zh[:], func=AF.Square)
                            teng = (nc.gpsimd if (wc + ch) % 2 == 0
                                    else nc.vector) if b == 0 else nc.vector
                            if ch == 0:
                                nc.vector.reduce_sum(s_sum[b][:, wc, :], zh[:], axis=AX.X)
                                gp_reduce48(s_sq[b][:, wc, :], sqh, teng)
                            else:
                                nc.vector.reduce_sum(s_t[:], zh[:], axis=AX.X)
                                nc.vector.tensor_add(s_sum[b][:, wc, :], s_sum[b][:, wc, :], s_t[:])
                                sq2 = stp.tile([90, H], F32, tag="st_tmp2", name="s_tmp2")
                                gp_reduce48(sq2[:], sqh, teng)
                                teng.tensor_add(s_sq[b][:, wc, :], s_sq[b][:, wc, :], sq2[:])
                    s_sumb = stp.tile([90, 2, H], BF16, tag=f"ssb{b}", name=f"ssumb{b}")
                    s_sqb = stp.tile([90, 2, H], BF16, tag=f"sqb{b}", name=f"ssqb{b}")
                    nc.vector.tensor_copy(s_sumb[:], s_sum[b][:])
                    nc.gpsimd.tensor_copy(s_sqb[:], s_sq[b][:])
                    nc.sync.dma_start(_stk(st_in[b], 0), s_sumb[:])
                    nc.sync.dma_start(_stk(st_in[b], 1), s_sqb[:])
                    nc.gpsimd.collective_compute(
                        "AllReduce", OP.add, replica_groups=rg,
                        ins=[st_in[b][:].opt()], outs=[st_out[b][:].opt()])

                for b in range(B):
                    s_sumr = stp.tile([90, 2, H], BF16, tag=f"ssr{b}", name=f"ssumr{b}")
                    s_sqr = stp.tile([90, 2, H], BF16, tag=f"sqr{b}", name=f"ssqr{b}")
                    nc.sync.dma_start(s_sumr[:], _stk(st_out[b], 0))
                    nc.sync.dma_start(s_sqr[:], _stk(st_out[b], 1))
                    nc.vector.tensor_scalar(out=s_m[b][:], in0=s_sumr[:],
                                            scalar1=1.0 / C, scalar2=None,
                                            op0=OP.mult)
                    nc.vector.tensor_scalar(out=s_r[b][:], in0=s_sqr[:],
                                            scalar1=1.0 / C, scalar2=None,
                                            op0=OP.mult)
                    tmp = stp.tile([90, 2, H], F32, tag=f"tmp{b}", name=f"tmpb{b}")
                    nc.vector.tensor_mul(tmp[:], s_m[b][:], s_m[b][:])
                    nc.vector.tensor_sub(s_r[b][:], s_r[b][:], tmp[:])
                    nc.scalar.activation(out=s_r[b][:], in_=s_r[b][:],
                                         func=AF.Sqrt, bias=c_eps[:90])
                    nc.vector.reciprocal(s_r[b][:], s_r[b][:])
                    # m*r rows for the rank-1 DFT correction of the LN mean
                    nc.vector.tensor_mul(s_mr[b][:], s_m[b][:], s_r[b][:])

                def _bcast48(t, b, wc):
                    # [90, H] stat slice broadcast along a trailing 48-axis
                    base = t[b][:]
                    return bass.AP(tensor=base.tensor,
                                   offset=base.offset + wc * H,
                                   ap=[[base.ap[0][0], 90], [1, H], [0, 48]])

                for b in range(B):
                    # ---- LN1 scale-only (z *= r, in place); the mean term
                    # is a channel-independent rank-1 correction applied at
                    # the F1 drain: yb = F1(z*r) - DFT_w(m*r).
                    yb = spA.tile([90, 92, BS], BF16, tag="YO", name="yb")
                    mrd = lnt.tile([90, 92], F32, tag="mrd", name="mrd")
                    pmr = pp.tile([90, 92], F32, tag="pp", name="psmr")
                    nc.tensor.matmul(pmr[:], s_mr[b][:, 0, :], c_fwp0[:],
                                     start=True, stop=False)
                    nc.tensor.matmul(pmr[:], s_mr[b][:, 1, :], c_fwp1[:],
                                     start=False, stop=True)
                    nc.vector.tensor_copy(mrd[:], pmr[:])
                    for ch in range(2):
                        mul_eng = nc.vector if ch == 0 else nc.gpsimd
                        for wc in range(2):
                            zt = zhs[(b, wc, ch)]
                            mul_eng.tensor_mul(zt[:], zt[:],
                                               _bcast48(s_r, b, wc))
                        zh0 = zhs[(b, 0, ch)]
                        zh1 = zhs[(b, 1, ch)]
                        # 5 output columns packed per PSUM bank -> 5x fewer
                        # (and 5x bigger) drains, fused with the -m*r fixup
                        for gi, g0 in enumerate(range(0, 48, 5)):
                            ncol = min(5, 48 - g0)
                            ps = pp.tile([90, 460], F32, tag="pp", name="psf1")
                            for idx in range(ncol):
                                cl_ = g0 + idx
                                sl = ps[:, idx * 92:(idx + 1) * 92]
                                nc.tensor.matmul(sl, zh0[:, :, cl_], c_fwp0[:],
                                                 start=True, stop=False)
                                nc.tensor.matmul(sl, zh1[:, :, cl_], c_fwp1[:],
                                                 start=False, stop=True)
                            c0 = ch * 48 + g0
                            src = bass.AP(tensor=ps[:].tensor,
                                          offset=ps[:].offset,
                                          ap=[[ps[:].ap[0][0], 90],
                                              [1, 92], [92, ncol]])
                            mrdv = bass.AP(tensor=mrd[:].tensor,
                                           offset=mrd[:].offset,
                                           ap=[[mrd[:].ap[0][0], 90],
                                               [1, 92], [0, ncol]])
                            nc.vector.tensor_sub(yb[:, :, c0:c0 + ncol],
                                                 src, mrdv)

                    # ---- F2 (5 wf packed per PSUM bank)
                    zb = spB.tile([BS, 2, KW, H], BF16, tag="ZO", name="zbt")
                    for g0 in range(0, KW, 5):
                        nwf = min(5, KW - g0)
                        prg = pp.tile([BS, 450], F32, tag="pp", name="psf2r")
                        pig = pp.tile([BS, 450], F32, tag="pp", name="psf2i")
                        for idx in range(nwf):
                            wf = g0 + idx
                            yr = yb[:, wf, :]
                            yi = yb[:, 46 + wf, :]
                            slr = prg[:, idx * 90:(idx + 1) * 90]
                            nc.tensor.matmul(slr, yr, c_fhc[:], start=True, stop=False)
                            nc.tensor.matmul(slr, yi, c_fhs[:], start=False, stop=True)
                            sli = pig[:, idx * 90:(idx + 1) * 90]
                            nc.tensor.matmul(sli, yi, c_fhc[:], start=True, stop=False)
                            nc.tensor.matmul(sli, yr, c_fhsm[:], start=False, stop=True)
                        nc.scalar.activation(out=zb[:, 0, g0:g0 + nwf, :],
                                             in_=prg[:, :nwf * 90],
                                             func=AF.Copy, scale=c_g1col[:])
                        nc.scalar.activation(out=zb[:, 1, g0:g0 + nwf, :],
                                             in_=pig[:, :nwf * 90],
                                             func=AF.Copy, scale=c_g1col[:])
                    nc.vector.tensor_scalar(out=zb[:, 0, 0, 0:1],
                                            in0=zb[:, 0, 0, 0:1],
                                            scalar1=c_spike[:], scalar2=None,
                                            op0=OP.add)

                    # ---- block MLP layer 1
                    o1 = spA.tile([BS + 1, 2, NM], BF16, tag="YO", name="o1t")
                    zr_f = zb[:, 0].rearrange("p a b -> p (a b)")
                    zi_f = zb[:, 1].rearrange("p a b -> p (a b)")
                    n0 = 0
                    while n0 < NM:
                        nn_ = min(512, NM - n0)
                        zr_s = zr_f[:, n0:n0 + nn_]
                        zi_s = zi_f[:, n0:n0 + nn_]
                        por = pp.tile([BS, 512], F32, tag="pp", name="pso1r")
                        nc.tensor.matmul(por[:, :nn_], c_w1r[:], zr_s,
                                         start=True, stop=False)
                        nc.tensor.matmul(por[:, :nn_], c_w1im[:], zi_s,
                                         start=False, stop=True)
                        poi = pp.tile([BS, 512], F32, tag="pp", name="pso1i")
                        nc.tensor.matmul(poi[:, :nn_], c_w1i[:], zr_s,
                                         start=True, stop=False)
                        nc.tensor.matmul(poi[:, :nn_], c_w1r[:], zi_s,
                                         start=False, stop=True)
                        nc.scalar.activation(out=o1[0:BS, 0, n0:n0 + nn_],
                                             in_=por[:, :nn_], func=AF.Relu,
                                             bias=c_b1r[:])
                        nc.scalar.activation(out=o1[0:BS, 1, n0:n0 + nn_],
                                             in_=poi[:, :nn_], func=AF.Relu,
                                             bias=c_b1i[:])
                        n0 += nn_

                    # ---- block MLP layer 2 + softshrink (5 wf packed/bank)
                    o2 = spB.tile([H, 2, KW, BS], BF16, tag="ZO", name="o2t")
                    o1r_f = o1[:, 0]
                    o1i_f = o1[:, 1]
                    for g0 in range(0, KW, 5):
                        nwf = min(5, KW - g0)
                        prg = pp.tile([H, 480], F32, tag="pp", name="pso2r")
                        pig = pp.tile([H, 480], F32, tag="pp", name="pso2i")
                        for idx in range(nwf):
                            wf = g0 + idx
                            lr = o1r_f[0:BS, wf * H:(wf + 1) * H]
                            li = o1i_f[0:BS, wf * H:(wf + 1) * H]
                            slr = prg[:, idx * BS:(idx + 1) * BS]
                            nc.tensor.matmul(slr, lr, c_w2a[0:BS, :], start=True, stop=False)
                            nc.tensor.matmul(slr, li, c_w2b[0:BS, :], start=False, stop=False)
                            nc.tensor.matmul(slr, c_ones1b[:, 0:H], c_b2rr[:], start=False, stop=True)
                            sli = pig[:, idx * BS:(idx + 1) * BS]
                            nc.tensor.matmul(sli, li, c_w2c[0:BS, :], start=True, stop=False)
                            nc.tensor.matmul(sli, lr, c_w2d[0:BS, :], start=False, stop=False)
                            nc.tensor.matmul(sli, c_ones1b[:, 0:H], c_b2ir[:], start=False, stop=True)
                        for ri, psm in ((0, prg), (1, pig)):
                            clip = clp.tile([H, 480], F32, tag="clip", name="clipt")
                            nc.vector.tensor_scalar(out=clip[:, :nwf * BS],
                                                    in0=psm[:, :nwf * BS],
                                                    scalar1=-LAM, scalar2=LAM,
                                                    op0=OP.max, op1=OP.min)
                            nc.vector.tensor_sub(o2[:, ri, g0:g0 + nwf, :],
                                                 psm[:, :nwf * BS],
                                                 clip[:, :nwf * BS])

                    # ---- inverse H-DFT -> u2r/u2i [46, (c, h)] (5 c / bank)
                    u2r = spQ.tile([KW, BS, H], BF16, tag="QU", name="u2rt")
                    u2i = spA.tile([KW, BS, H], BF16, tag="YO", name="u2it")
                    for g0 in range(0, BS, 5):
                        ncl = min(5, BS - g0)
                        purg = pp.tile([KW, 450], F32, tag="pp", name="psur")
                        puig = pp.tile([KW, 450], F32, tag="pp", name="psui")
                        for idx in range(ncl):
                            c = g0 + idx
                            lr = o2[:, 0, :, c]
                            li = o2[:, 1, :, c]
                            slr = purg[:, idx * H:(idx + 1) * H]
                            nc.tensor.matmul(slr, lr, c_fhc[:], start=True, stop=False)
                            nc.tensor.matmul(slr, li, c_fhsm[:], start=False, stop=True)
                            sli = puig[:, idx * H:(idx + 1) * H]
                            nc.tensor.matmul(sli, li, c_fhc[:], start=True, stop=False)
                            nc.tensor.matmul(sli, lr, c_fhs[:], start=False, stop=True)
                        nc.scalar.activation(out=u2r[:, g0:g0 + ncl, :],
                                             in_=purg[:, :ncl * H], func=AF.Copy)
                        nc.vector.tensor_copy(u2i[:, g0:g0 + ncl, :],
                                              puig[:, :ncl * H])

                    # ---- inverse W-DFT -> SBUF gather sbA (bf16), 2 c/bank
                    sbA = sbg.tile([H, BS, W], BF16, tag="sbA", name="sbA")
                    for c0 in range(0, BS, 2):
                        pf = pp.tile([H, 360], F32, tag="pp", name="psf")
                        for idx in range(2):
                            c = c0 + idx
                            sl = pf[:, idx * W:(idx + 1) * W]
                            nc.tensor.matmul(sl, u2r[:, c, :], c_iwrt[:],
                                             start=True, stop=False)
                            nc.tensor.matmul(sl, u2i[:, c, :], c_iwit[:],
                                             start=False, stop=True)
                        if (c0 // 2) % 2 == 0:
                            nc.scalar.activation(out=sbA[:, c0:c0 + 2, :],
                                                 in_=pf[:], func=AF.Copy)
                        else:
                            nc.vector.tensor_copy(sbA[:, c0:c0 + 2, :], pf[:])

                    # ---- a2a send pieces (SBUF -> DRAM), quarter-major
                    # so quarter 0's collective can fire first
                    for q in range(4):
                        qw = qws[q]
                        for jq in range(4):
                            j = b * 4 + jq
                            for (ha, hb_, wa, wb_, toff) in _shard_pieces(
                                    jq, QB[q], QB[q + 1]):
                                src = sbA[ha:hb_, :, wa:wb_]
                                dst = bass.AP(
                                    tensor=a2a_in[q][:].tensor,
                                    offset=a2a_in[q][:].offset
                                    + (j * BS * qw + toff),
                                    ap=[[wb_ - wa, hb_ - ha], [qw, BS],
                                        [1, wb_ - wa]])
                                nc.sync.dma_start(dst, src)

            for q in range(4):
                nc.gpsimd.collective_compute(
                    "AllToAll", OP.bypass, replica_groups=rg,
                    ins=[a2a_in[q][:].opt()], outs=[a2a_out[q][:].opt()])

            # ================= phase 2 =================
            with ExitStack() as _st2:
                fc1p = _st2.enter_context(tc.tile_pool(name="fc1p", bufs=1))
                fc2p = _st2.enter_context(tc.tile_pool(name="fc2p", bufs=1))
                lnp = _st2.enter_context(tc.tile_pool(name="lnp", bufs=1))

                FP8 = mybir.dt.float8e4
                c_fc1 = [fc1p.tile([128, 2, HID], FP8, tag=f"fc1_{i}",
                                   name=f"cfc1_{i}") for i in range(3)]
                for i in range(3):
                    nc.sync.dma_start(c_fc1[i][:], fc1q[i])
                c_fc2 = [fc2p.tile([128, 2, C], FP8, tag=f"fc2_{i}",
                                   name=f"cfc2_{i}") for i in range(12)]
                for i in range(12):
                    nc.sync.dma_start(c_fc2[i][:], fc2q[i])

                R1B = lnp.tile([128, TSH], BF16, tag="R1B", name="R1B")
                MR1B = lnp.tile([128, TSH], BF16, tag="MR1B", name="MR1B")

                with ExitStack() as _stR:
                    rowp = _stR.enter_context(tc.tile_pool(name="rowp", bufs=1))
                    pbR = _stR.enter_context(
                        tc.tile_pool(name="pbR", bufs=2, space="PSUM"))
                    # all-shard LN1 stats rows [8, 4050]
                    r1_8 = rowp.tile([NCORES, TSH], F32, tag="r18", name="r18")
                    mr1_8 = rowp.tile([NCORES, TSH], F32, tag="mr18", name="mr18")
                    rtmp = rowp.tile([NCORES, TSH], F32, tag="rtmp", name="rtmp")
                    r1_8b = rowp.tile([NCORES, TSH], BF16, tag="r18b", name="r18b")
                    rtmpb = rowp.tile([NCORES, TSH], BF16, tag="rtmpb", name="rtmpb")
                    for kind, dstt in ((0, rtmpb), (1, r1_8b)):
                        for s in range(NCORES):
                            bb, jq = s // 4, s % 4
                            for (ha, hb_, wa, wb_, toff) in _shard_pieces(jq):
                                src_ = bass.AP(
                                    tensor=st_out[bb][:].tensor,
                                    offset=st_out[bb][:].offset
                                    + (kind * W * H + wa * H + ha),
                                    ap=[[0, 1], [1, hb_ - ha], [H, wb_ - wa]])
                                nc.sync.dma_start(
                                    dstt[s:s + 1,
                                         toff:toff + (hb_ - ha) * (wb_ - wa)],
                                    src_)
                    nc.vector.tensor_scalar(out=mr1_8[:], in0=rtmpb[:],
                                            scalar1=1.0 / C, scalar2=None,
                                            op0=OP.mult)           # m1
                    nc.vector.tensor_scalar(out=r1_8[:], in0=r1_8b[:],
                                            scalar1=1.0 / C, scalar2=None,
                                            op0=OP.mult)           # q/C
                    nc.vector.tensor_mul(rtmp[:], mr1_8[:], mr1_8[:])
                    nc.vector.tensor_sub(r1_8[:], r1_8[:], rtmp[:])  # var
                    nc.scalar.activation(out=r1_8[:], in_=r1_8[:], func=AF.Sqrt,
                                         bias=c_eps[:NCORES])
                    nc.vector.reciprocal(r1_8[:], r1_8[:])           # r1
                    nc.vector.tensor_mul(mr1_8[:], mr1_8[:], r1_8[:])  # m1*r1

                    # pre-broadcast this core's r1 / m1*r1 to bf16 planes
                    for it, T in enumerate(TTS):
                        t0 = TT0[it]
                        for rows, plane in ((r1_8, R1B), (mr1_8, MR1B)):
                            pb = pbR.tile([128, 512], F32, tag="pbc")
                            nc.tensor.matmul(pb[:, :T], c_mask[:],
                                             rows[:, t0:t0 + T],
                                             start=True, stop=True)
                            nc.scalar.activation(out=plane[:, t0:t0 + T],
                                                 in_=pb[:, :T], func=AF.Copy)

                xcp = _st2.enter_context(tc.tile_pool(name="xcp", bufs=6))
                rcvp = _st2.enter_context(tc.tile_pool(name="rcvp", bufs=6))
                t1p = _st2.enter_context(tc.tile_pool(name="t1p", bufs=3))
                htokp = _st2.enter_context(tc.tile_pool(name="htokp", bufs=12))
                htnp = _st2.enter_context(tc.tile_pool(name="htnp", bufs=12))
                hidp = _st2.enter_context(tc.tile_pool(name="hidp", bufs=1))
                rw2 = _st2.enter_context(tc.tile_pool(name="rw2", bufs=2))
                bcp = _st2.enter_context(tc.tile_pool(name="bcp", bufs=2))
                outp = _st2.enter_context(tc.tile_pool(name="outp", bufs=3))
                ph = _st2.enter_context(tc.tile_pool(name="ph", bufs=2, space="PSUM"))
                po = _st2.enter_context(tc.tile_pool(name="po", bufs=2, space="PSUM"))
                pst = _st2.enter_context(tc.tile_pool(name="pst", bufs=2, space="PSUM"))
                pbc = _st2.enter_context(tc.tile_pool(name="pbc", bufs=2, space="PSUM"))

                # software-pipelined chunk loop: front half (loads, assemble,
                # LN2 stats, normalize) runs one chunk ahead of the back half
                # (fc1 -> gelu -> fc2 -> residual -> store).
                hts = {}
                htns = {}
                hid = hidp.tile([128, NMO, 512], FP8, tag="hid", name="hid")
                for i in range(NCH + 1):
                    if i < NCH:
                        T = TTS[i]
                        t0 = TT0[i]
                        htoks = []
                        htnl = []
                        for cc in range(NCC):
                            xct = xcp.tile([128, 512], F32, tag="xct")
                            nc.sync.dma_start(xct[:, :T],
                                              xc[cc * 128:(cc + 1) * 128, t0:t0 + T])
                            rcv = rcvp.tile([128, 512], BF16, tag="rcv")
                            q = QMAP[i]
                            lt0 = t0 - QB[q]
                            c0 = cc * 128
                            r0 = 0
                            while r0 < 128:
                                s_blk = (c0 + r0) // BS
                                c_in = (c0 + r0) % BS
                                nrow = min(BS - c_in, 128 - r0)
                                nc.sync.dma_start(
                                    rcv[r0:r0 + nrow, :T],
                                    a2a_out[q][s_blk, c_in:c_in + nrow,
                                               lt0:lt0 + T])
                                r0 += nrow
                            ht = htokp.tile([128, 512], F32, tag="htok")
                            htoks.append(ht)
                            t1 = t1p.tile([128, 512], F32, tag="t1")
                            nc.gpsimd.tensor_mul(t1[:, :T], xct[:, :T],
                                                 R1B[:, t0:t0 + T])
                            nc.vector.tensor_sub(t1[:, :T], t1[:, :T],
                                                 MR1B[:, t0:t0 + T])
                            nc.vector.tensor_scalar(out=t1[:, :T], in0=t1[:, :T],
                                                    scalar1=c_g1f[:, cc:cc + 1],
                                                    scalar2=c_be1f[:, cc:cc + 1],
                                                    op0=OP.mult, op1=OP.add)
                            nc.gpsimd.tensor_add(ht[:, :T], rcv[:, :T], xct[:, :T])
                            nc.vector.tensor_add(ht[:, :T], ht[:, :T], t1[:, :T])

                        # LN2 stats via bf16 ones-matmul (1 cyc/row vs 4)
                        ps_s = pst.tile([1, 512], F32, tag="pst")
                        ps_q = pst.tile([1, 512], F32, tag="pst")
                        htbs = []
                        for cc in range(NCC):
                            htb = t1p.tile([128, 512], BF16, tag="htb", name="htb")
                            nc.scalar.activation(out=htb[:, :T],
                                                 in_=htoks[cc][:, :T],
                                                 func=AF.Copy)
                            htbs.append(htb)
                        for cc in range(NCC):
                            nc.tensor.matmul(ps_s[:, :T], c_ones128b[:],
                                             htbs[cc][:, :T],
                                             start=(cc == 0), stop=(cc == NCC - 1))
                        hsqs = []
                        for cc in range(NCC):
                            hsq = t1p.tile([128, 512], BF16, tag="hsq", name="hsq")
                            heng = nc.gpsimd if cc % 2 == 0 else nc.vector
                            heng.tensor_mul(hsq[:, :T], htbs[cc][:, :T],
                                            htbs[cc][:, :T])
                            hsqs.append(hsq)
                        for cc in range(NCC):
                            nc.tensor.matmul(ps_q[:, :T], c_ones128b[:],
                                             hsqs[cc][:, :T],
                                             start=(cc == 0), stop=(cc == NCC - 1))
                        m2r = rw2.tile([1, 512], F32, tag="m2r")
                        r2r = rw2.tile([1, 512], F32, tag="r2r")
                        vv = rw2.tile([1, 512], F32, tag="vv")
                        nc.vector.tensor_scalar(out=m2r[:, :T], in0=ps_s[:, :T],
                                                scalar1=1.0 / C, scalar2=None,
                                                op0=OP.mult)
                        nc.vector.tensor_scalar(out=r2r[:, :T], in0=ps_q[:, :T],
                                                scalar1=1.0 / C, scalar2=None,
                                                op0=OP.mult)
                        nc.vector.tensor_mul(vv[:, :T], m2r[:, :T], m2r[:, :T])
                        nc.vector.tensor_sub(r2r[:, :T], r2r[:, :T], vv[:, :T])
                        nc.scalar.activation(out=r2r[:, :T], in_=r2r[:, :T],
                                             func=AF.Sqrt, bias=c_eps[:1])
                        nc.vector.reciprocal(r2r[:, :T], r2r[:, :T])
                        # broadcast m2, r2 to all partitions
                        m2b = bcp.tile([128, 512], F32, tag="m2b")
                        r2b = bcp.tile([128, 512], F32, tag="r2b")
                        for rowt, bt in ((m2r, m2b), (r2r, r2b)):
                            pb = pbc.tile([128, 512], F32, tag="pbc")
                            nc.tensor.matmul(pb[:, :T], c_ones1[:], rowt[:, :T],
                                             start=True, stop=True)
                            nc.scalar.activation(out=bt[:, :T], in_=pb[:, :T],
                                                 func=AF.Copy)
                        # normalize -> fp8e4m3 (DoubleRow slot layout)
                        for cc in range(NCC):
                            if cc % 2 == 0:
                                htn = htnp.tile([128, 2, 512], FP8, tag="htn")
                                htnl.append(htn)
                            tn = t1p.tile([128, 512], F32, tag="t1", name="tn")
                            nc.vector.tensor_sub(tn[:, :T], htoks[cc][:, :T],
                                                 m2b[:, :T])
                            nc.vector.tensor_mul(htn[:, cc % 2, :T], tn[:, :T],
                                                 r2b[:, :T])
                        hts[i] = htoks
                        htns[i] = htnl

                    if i >= 1:
                        j = i - 1
                        T = TTS[j]
                        t0 = TT0[j]
                        htoks = hts.pop(j)
                        htnl = htns.pop(j)
                        # fc1 (fp8 DoubleRow) + gelu -> hid (fp8)
                        for mo in range(NMO):
                            php = ph.tile([128, 512], F32, tag="ph")
                            for p_ in range(3):
                                nc.tensor.matmul(
                                    php[:, :T],
                                    c_fc1[p_][:, :, mo * 128:(mo + 1) * 128],
                                    htnl[p_][:, :, :T],
                                    start=(p_ == 0), stop=(p_ == 2),
                                    perf_mode=mybir.MatmulPerfMode.DoubleRow)
                            nc.scalar.activation(out=hid[:, mo, :T],
                                                 in_=php[:, :T], func=AF.Gelu,
                                                 scale=1.0 / WSCALE,
                                                 bias=c_gbias[:, mo:mo + 1])
                        # fc2 (fp8 DoubleRow) + bias + residual -> store
                        for co in range(NCC):
                            pop = po.tile([128, 512], F32, tag="po")
                            for p_ in range(12):
                                nc.tensor.matmul(
                                    pop[:, :T],
                                    c_fc2[p_][:, :, co * 128:(co + 1) * 128],
                                    hid[:, 2 * p_:2 * p_ + 2, :T],
                                    start=(p_ == 0), stop=(p_ == 11),
                                    perf_mode=mybir.MatmulPerfMode.DoubleRow)
                            osb = outp.tile([128, 512], F32, tag="osb")
                            nc.scalar.activation(out=osb[:, :T],
                                                 in_=pop[:, :T],
                                                 func=AF.Identity,
                                                 scale=1.0 / WSCALE,
                                                 bias=c_fc2b[:, co:co + 1])
                            nc.vector.tensor_add(osb[:, :T], osb[:, :T],
                                                 htoks[co][:, :T])
                            nc.sync.dma_start(
                                out[co * 128:(co + 1) * 128, t0:t0 + T],
                                osb[:, :T])

    nc.compile()
    _CACHE["nc"] = nc
    return nc


def _host_prep(inputs):
    x = np.ascontiguousarray(np.asarray(inputs["x"], dtype=np.float32))
    g1 = np.asarray(inputs["g1"], np.float32); be1 = np.asarray(inputs["be1"], np.float32)
    g2 = np.asarray(inputs["g2"], np.float32); be2 = np.asarray(inputs["be2"], np.float32)
    w1 = np.asarray(inputs["w1"], np.float32); b1 = np.asarray(inputs["b1"], np.float32)
    w2 = np.asarray(inputs["w2"], np.float32); b2 = np.asarray(inputs["b2"], np.float32)
    fc1_w = np.asarray(inputs["fc1_w"], np.float32)
    fc1_b = np.asarray(inputs["fc1_b"], np.float32)
    fc2_w = np.asarray(inputs["fc2_w"], np.float32)
    fc2_b = np.asarray(inputs["fc2_b"], np.float32)

    fwp0, fwp1, fhc_m, fhs_m, fhsm_m, iwrt_m, iwit_m = _dft_consts()
    xf = x.reshape(TOK, C)
    bf = ml_dtypes.bfloat16
    f8 = ml_dtypes.float8_e4m3fn
    fc1q_m = np.ascontiguousarray(
        (g2[:, None] * fc1_w * WSCALE).reshape(3, 2, 128, HID)
        .transpose(0, 2, 1, 3)).astype(f8)                     # (3,128,2,3072)
    fc2q_m = np.ascontiguousarray(
        (fc2_w * WSCALE).reshape(12, 2, 128, C)
        .transpose(0, 2, 1, 3)).astype(f8)                     # (12,128,2,768)
    gbias_v = (fc1_b + be2 @ fc1_w).astype(np.float32)         # (3072,)
    gbias_m = np.ascontiguousarray(gbias_v.reshape(NMO, 128).T)  # (128, 24)
    fc2b_m = np.ascontiguousarray(fc2_b.reshape(NCC, 128).T)
    g1f_m = np.ascontiguousarray(g1.reshape(NCC, 128).T)
    be1f_m = np.ascontiguousarray(be1.reshape(NCC, 128).T)
    ones1 = np.ones((1, 128), np.float32)
    ones1b = np.ones((1, 128), bf)
    ones128 = np.ones((128, 1), np.float32)
    ones128b = np.ones((128, 1), bf)

    in_maps = []
    for k in range(NCORES):
        ck = slice(k * BS, (k + 1) * BS)
        # (W, B, H, 96) -> (W, B, 2, H, 48), bf16
        xw_k = np.ascontiguousarray(
            x[:, :, :, ck].transpose(2, 0, 1, 3)
            .reshape(W, B, H, 2, 48).transpose(0, 1, 3, 2, 4)).astype(bf)
        xc_k = np.ascontiguousarray(xf[k * TSH:(k + 1) * TSH, :].T)
        w1r_k = np.ascontiguousarray(w1[k, :, :, 0])
        w1i_k = np.ascontiguousarray(w1[k, :, :, 1])
        w2r_k = np.ascontiguousarray(w2[k, :, :, 0])
        w2i_k = np.ascontiguousarray(w2[k, :, :, 1])
        b2r_k = b2[k, :, 0]; b2i_k = b2[k, :, 1]
        zr = np.zeros((1, BS), np.float32)
        mask = np.zeros((NCORES, 128), np.float32); mask[k, :] = 1.0
        in_maps.append({
            "xw": xw_k, "xc": xc_k,
            "fwp0": fwp0.astype(bf), "fwp1": fwp1.astype(bf),
            "fhc": fhc_m.astype(bf), "fhs": fhs_m.astype(bf),
            "fhsm": fhsm_m.astype(bf),
            "iwrt": iwrt_m.astype(bf), "iwit": iwit_m.astype(bf),
            "w1r": w1r_k.astype(bf), "w1i": w1i_k.astype(bf),
            "w1im": (-w1i_k).astype(bf),
            "b1r": b1[k, :, 0:1].copy(), "b1i": b1[k, :, 1:2].copy(),
            "w2a": np.concatenate([w2r_k, b2r_k[None, :]], 0).astype(bf),
            "w2b": np.concatenate([-w2i_k, zr], 0).astype(bf),
            "w2c": np.concatenate([w2r_k, zr], 0).astype(bf),
            "w2d": np.concatenate([w2i_k, b2i_k[None, :]], 0).astype(bf),
            "g1col": g1[ck][:, None].copy(),
            "b2rr": b2r_k[None, :].astype(bf), "b2ir": b2i_k[None, :].astype(bf),
            "spike": (be1[ck] * SQN)[:, None].astype(np.float32),
            "fc1q": fc1q_m, "fc2q": fc2q_m, "gbias": gbias_m,
            "fc2b": fc2b_m, "g1f": g1f_m, "be1f": be1f_m,
            "ones1": ones1, "ones1b": ones1b, "ones128": ones128,
            "ones128b": ones128b,
            "mask128": mask,
        })
    return in_maps


def kernel(**inputs):
    nc = _build_nc()
    in_maps = _host_prep(inputs)
    res = run_bass_kernel_spmd(nc, in_maps, core_ids=list(range(NCORES)))
    outs = [np.asarray(res.results[j]["out"], dtype=np.float32).T
            for j in range(NCORES)]
    full = np.concatenate(outs, axis=0).reshape(B, H, W, C)
    return np.ascontiguousarray(full, dtype=np.float32)
